# revision 29
# baseline (speedup 1.0000x reference)
"""AR video patch transformer forward on 8 Trainium2 NeuronCores.

Strategy: pure data parallelism — each core runs the full 8-layer
transformer on one batch element. Host does patchify/unpatchify and
weight preprocessing (scale folds, padding, lhsT tiling, fp16 cast).

v2 vs v1:
 - Emission reordered so the tensor queue stays dense (HAM clock-gate
   stays at 8/8 instead of oscillating to half clock in attention).
 - Attention softmax denominators: DVE reciprocal_approx_fast instead of
   a scalar Ln -> table swap -> Exp chain (removes a ~6.4us/layer stall).
 - RoPE pair-swap via DVE stream_shuffle (was a PE matmul).
 - All scalar rsqrt via Ln+Exp; a post-compile pass rewrites activation
   table-set ids to the combined natural_log_exp set and drops redundant
   loads (74 -> ~18 table loads).
 - Aux matmuls 2-way packed on the PE array (msq q/k, rope alpha bcast).
 - Fused psum-consume ops on DVE (scalar_tensor_tensor) for residual
   adds and gate*up products.

v3 vs v2:
 - MLP gate/up psums consumed immediately after each accumulation group
   (safe under a 2-slot psum rotation; keeps the PE queue dense).
 - r2/rf broadcast staged to fp16 once, so the 8 h2/hN row scalings run
   at 2x DVE rate instead of re-reading the fp32 psum broadcast.
 - Known-dead-end notes: fusing the per-kt softmax exps via packed score
   psums ([128,1024] 2-bank or single-bank layouts) compiles + passes
   CoreSim but faults at runtime on HW; GpSimd cannot read PSUM (BIR
   verifier); DVE reciprocal custom ops require SBUF fp32 in/out;
   batching the den Ln/Exp across ti-pairs regresses ~190us (stalls the
   attn-v pipeline).  fp8 is out of error budget (absmax/rms gate 2e-2,
   fp16 sits at 4e-3, one e4m3 GEMM costs ~3%).
"""

import numpy as np

import concourse.bass as bass
import concourse.mybir as mybir
from concourse import bacc
from concourse.tile import TileContext
from concourse.bass_utils import run_bass_kernel_spmd

F = mybir.ActivationFunctionType
ALU = mybir.AluOpType
FP16 = mybir.dt.float16
FP32 = mybir.dt.float32

# Model config (hardcoded from the problem spec)
B = 8; T = 8; C = 3; RES = 64; P = 8
D = 1024; NH = 16; HD = 64; NL = 8
INNER = 2730
NP_ = 64           # patches per frame
PD = 192           # patch dim
PDP = 256          # padded patch dim (2 k-tiles)
L = 512            # tokens
EPS = 1e-6
KT = D // 128      # 8
IH = INNER // 2    # 1365 half-inner
IHP = 1408         # padded half-inner (11 tiles)
JT = IHP // 128    # 11
EP9 = 2.0 ** -9    # exact fp16 scalar used for the eps matmul

N_CORES = 8
_CACHE = {}

import os
DBG_NO_TBLFIX = os.environ.get("K_NO_TBLFIX", "") == "1"
DBG_NO_SHUF = os.environ.get("K_NO_SHUF", "") == "1"
DBG_NO_RECIP = os.environ.get("K_NO_RECIP", "") == "1"
# CoreSim has no Silu; K_SIMSAFE swaps in Sigmoid (structure-identical)
SIM_SAFE = os.environ.get("K_SIMSAFE", "") == "1"

SWAP_MASK = [i ^ 1 for i in range(32)]

# activation table sets (trn2/cayman act_info.json order)
SET_NATLOG_EXP = 6     # ln, exp, square, copy, ...
_REWRITE_SETS = {0, 5}  # exp_and_others / natural_log -> natlog_exp


class _Bacc(bacc.Bacc):
    """Bacc with a post-pass that merges ln/exp table sets and deletes
    redundant table loads (the stock pass picks the first set containing
    each function, so ln<->exp sequences thrash)."""

    def insert_act_table_loads(self):
        super().insert_act_table_loads()
        if DBG_NO_TBLFIX:
            return
        for blk in self.main_func.blocks:
            resident = None
            keep = []
            for inst in blk.instructions:
                if isinstance(inst, mybir.InstLoadActFuncSet):
                    if inst.act_func_set_id in _REWRITE_SETS:
                        inst.act_func_set_id = SET_NATLOG_EXP
                    if inst.act_func_set_id == resident and not inst.sync_info:
                        continue  # redundant, drop
                    resident = inst.act_func_set_id
                keep.append(inst)
            blk.instructions[:] = keep


# ----------------------------------------------------------------------
# host-side preprocessing
# ----------------------------------------------------------------------

def _lhsT_tile(w):
    """[Din, Dout] -> [Dout/128, 128, Din] fp16 lhsT-tiled blocks."""
    din, dout = w.shape
    kt, ot = din // 128, dout // 128
    return np.ascontiguousarray(
        w.reshape(kt, 128, ot, 128).transpose(2, 1, 0, 3).reshape(ot, 128, din)
    ).astype(np.float16)


def _rope_tables(scale):
    """C/S tables [128, L] with the per-dim norm scale folded in
    (scale applied before rotation, matching the reference order)."""
    q = HD // 4  # 16
    inv = 1.0 / (10000.0 ** (np.arange(q, dtype=np.float64) / q))
    t_idx = np.repeat(np.arange(T), NP_)
    s_idx = np.tile(np.arange(NP_), T)
    ang = np.concatenate(
        [t_idx[:, None] * inv[None, :], s_idx[:, None] * inv[None, :]], axis=1
    )  # (L, 32)
    cdm = np.zeros((128, L), np.float64)
    sdm = np.zeros((128, L), np.float64)
    for d in range(128):
        dl = d % 64
        i = dl // 2
        cdm[d] = np.cos(ang[:, i]) * scale[dl]
        sg = -1.0 if d % 2 == 0 else 1.0
        sdm[d] = sg * np.sin(ang[:, i]) * scale[dl ^ 1]
    return cdm.astype(np.float16), sdm.astype(np.float16)


def _prep_weights(inp):
    w = {}
    n1 = inp["norm1_scale"]; n2 = inp["norm2_scale"]
    qk_t = np.empty((NL, 16, 128, D), np.float16)
    wv_p = np.empty((NL, KT, 128, D), np.float16)
    wo_t = np.empty((NL, 8, 128, D), np.float16)
    gu_t = np.empty((NL, 44, 128, D), np.float16)
    dn_t = np.empty((NL, 8, 128, IHP), np.float16)
    for l in range(NL):
        w1 = inp["qkv_w"][l] * n1[l][:, None]
        qk_t[l] = _lhsT_tile(w1[:, :2048])
        wv_p[l] = w1[:, 2048:].reshape(KT, 128, D).astype(np.float16)
        wo_t[l] = _lhsT_tile(inp["out_w"][l])
        g = inp["gate_w"][l] * n2[l][:, None]
        u = inp["up_w"][l] * n2[l][:, None]
        gp = np.zeros((D, 2 * IHP), np.float32)
        up = np.zeros((D, 2 * IHP), np.float32)
        gp[:, :IH] = g[:, :IH]; gp[:, IHP:IHP + IH] = g[:, IH:]
        up[:, :IH] = u[:, :IH]; up[:, IHP:IHP + IH] = u[:, IH:]
        gt = _lhsT_tile(gp); ut = _lhsT_tile(up)
        order = []
        for j in range(JT):
            order += [gt[j], gt[JT + j], ut[j], ut[JT + j]]
        gu_t[l] = np.stack(order)
        dp = np.zeros((IHP, D), np.float32)
        dp[:IH] = inp["down_w"][l]
        dn_t[l] = _lhsT_tile(dp)
    w["qk_t"] = qk_t; w["wv_p"] = wv_p; w["wo_t"] = wo_t
    w["gu_t"] = gu_t; w["dn_t"] = dn_t

    pe = np.zeros((PDP, D), np.float32)
    pe[:PD] = inp["patch_embed_w"]
    w["pe_t"] = _lhsT_tile(pe)
    hw = np.zeros((D, PDP), np.float32)
    hw[:, :PD] = inp["head_w"] * inp["normf_scale"][:, None]
    w["hd_t"] = _lhsT_tile(hw)

    # per-head broadcast matrix (block ones); q rows 0-15, k rows 32-47
    e1 = np.zeros((48, 8, 128), np.float16)
    for t in range(8):
        for dl in range(128):
            e1[2 * t + dl // 64, t, dl] = 1.0
            e1[32 + 2 * t + dl // 64, t, dl] = 1.0
    w["e1"] = np.ascontiguousarray(e1.reshape(48, 8 * 128))

    w["esc"] = inp["embed_norm_scale"].reshape(1, 8 * 128).astype(np.float16)

    cq = np.empty((NL, 128, L), np.float16); sq = np.empty((NL, 128, L), np.float16)
    ck = np.empty((NL, 128, L), np.float16); sk = np.empty((NL, 128, L), np.float16)
    for l in range(NL):
        cq[l], sq[l] = _rope_tables(inp["q_norm_scale"][l])
        ck[l], sk[l] = _rope_tables(inp["k_norm_scale"][l])
    w["cq"] = cq; w["sq"] = sq; w["ck"] = ck; w["sk"] = sk
    b16 = np.zeros((128, 8 * 16), np.float16)
    for t in range(8):
        for dl in range(128):
            b16[dl, t * 16 + 2 * t + dl // 64] = 1.0 / 64.0
    w["b16"] = b16
    psw = np.zeros((128, 128), np.float16)
    for i in range(128):
        psw[i ^ 1, i] = 1.0
    w["psw"] = psw
    return w


def _patchify(frames_b):
    # (T, C, RES, RES) -> (L, PD)
    h = RES // P
    x = frames_b.reshape(T, C, h, P, h, P)
    x = x.transpose(0, 2, 4, 1, 3, 5).reshape(T * h * h, C * P * P)
    return x


def _unpatchify(tokens):
    # (L, PD) -> (T, C, RES, RES)
    h = RES // P
    y = tokens.reshape(T, h, h, C, P, P)
    return y.transpose(0, 3, 1, 4, 2, 5).reshape(T, C, RES, RES)


# ----------------------------------------------------------------------
# device kernel
# ----------------------------------------------------------------------

def _build(nl=NL):
    nc = _Bacc()
    d = {}
    d["x0T"] = nc.dram_tensor("x0T", [PDP, L], FP16, kind="ExternalInput")
    d["qk_t"] = nc.dram_tensor("qk_t", [NL, 16, 128, D], FP16, kind="ExternalInput")
    d["wv_p"] = nc.dram_tensor("wv_p", [NL, KT, 128, D], FP16, kind="ExternalInput")
    d["wo_t"] = nc.dram_tensor("wo_t", [NL, 8, 128, D], FP16, kind="ExternalInput")
    d["gu_t"] = nc.dram_tensor("gu_t", [NL, 44, 128, D], FP16, kind="ExternalInput")
    d["dn_t"] = nc.dram_tensor("dn_t", [NL, 8, 128, IHP], FP16, kind="ExternalInput")
    d["pe_t"] = nc.dram_tensor("pe_t", [8, 128, PDP], FP16, kind="ExternalInput")
    d["hd_t"] = nc.dram_tensor("hd_t", [2, 128, D], FP16, kind="ExternalInput")
    d["e1"] = nc.dram_tensor("e1", [48, 8 * 128], FP16, kind="ExternalInput")
    d["esc"] = nc.dram_tensor("esc", [1, 8 * 128], FP16, kind="ExternalInput")
    d["cq"] = nc.dram_tensor("cq", [NL, 128, L], FP16, kind="ExternalInput")
    d["sq"] = nc.dram_tensor("sq", [NL, 128, L], FP16, kind="ExternalInput")
    d["ck"] = nc.dram_tensor("ck", [NL, 128, L], FP16, kind="ExternalInput")
    d["sk"] = nc.dram_tensor("sk", [NL, 128, L], FP16, kind="ExternalInput")
    d["b16"] = nc.dram_tensor("b16", [128, 8 * 16], FP16, kind="ExternalInput")
    d["psw"] = nc.dram_tensor("psw", [128, 128], FP16, kind="ExternalInput")
    out_d = nc.dram_tensor("out", [PD, L], FP32, kind="ExternalOutput")

    with TileContext(nc) as tc:
        _emit(nc, tc, d, out_d, nl)
    nc.compile()
    return nc


def _emit(nc, tc, d, out_d, nl):
    import contextlib
    ctx = contextlib.ExitStack()
    with ctx:
        cpool = ctx.enter_context(tc.tile_pool(name="consts", bufs=1))
        xpool = ctx.enter_context(tc.tile_pool(name="x", bufs=1))
        wpool = ctx.enter_context(tc.tile_pool(name="w", bufs=3))
        apool = ctx.enter_context(tc.tile_pool(name="act", bufs=1))
        spool = ctx.enter_context(tc.tile_pool(name="small", bufs=2))
        # psum: tag "g" slots are 2 banks wide ([128,1024] fp32) so the
        # attention scores for one head-pair half (kt0+kt1+kt3 packed) fit
        # one slot; general [128,512] psums use half a slot.  2(g)*2 + 2(acc)
        # + 2(bc) = 8 banks exactly.
        ps_g = ctx.enter_context(tc.tile_pool(name="psg", bufs=4, space="PSUM"))
        ps_acc = ctx.enter_context(tc.tile_pool(name="psacc", bufs=2, space="PSUM"))
        ps_bc = ctx.enter_context(tc.tile_pool(name="psbc", bufs=2, space="PSUM"))

        # ---- persistent constants ----
        b16 = cpool.tile([128, 128], FP16, name="b16")
        nc.sync.dma_start(b16[:], d["b16"][:])
        pswc = cpool.tile([128, 128], FP16, name="pswc")
        nc.sync.dma_start(pswc[:], d["psw"][:])
        e1s = cpool.tile([48, 8 * 128], FP16, name="e1s")
        nc.sync.dma_start(e1s[:], d["e1"][:])
        escs = cpool.tile([1, 8 * 128], FP16, name="escs")
        nc.sync.dma_start(escs[:], d["esc"][:])
        ones1 = cpool.tile([1, 128], FP16, name="ones1")
        nc.gpsimd.memset(ones1[:], 1.0)
        o1c = cpool.tile([33, 64], FP16, name="o1c")
        nc.gpsimd.memset(o1c[:], 1.0)
        onesd = cpool.tile([128, 1], FP16, name="onesd")
        nc.gpsimd.memset(onesd[:], 1.0 / 1024.0)
        ep9 = cpool.tile([1, 16], FP16, name="ep9")
        nc.gpsimd.memset(ep9[:], EP9)
        epsb = cpool.tile([128, 1], FP32, name="epsb")
        nc.gpsimd.memset(epsb[:], EPS)
        identf = cpool.tile([1, 1], FP32, name="identf")
        nc.gpsimd.memset(identf[:], 1.0)

        # residual stream
        xs = [xpool.tile([128, L], FP32, name=f"x{t}") for t in range(8)]
        x16 = [xpool.tile([128, L], FP16, name=f"m{t}") for t in range(8)]

        def rms_recip(mean_ps, name, dtype=FP16):
            """[1, L] psum mean -> [1, L] rsqrt(mean+eps) via Ln+Exp."""
            lnm = spool.tile([1, L], FP32, name=f"lnm_{name}", tag="r32")
            nc.scalar.activation(lnm[:], mean_ps[:], F.Ln, bias=epsb[0:1])
            r = spool.tile([1, L], dtype, name=f"r_{name}", tag="r16")
            nc.scalar.activation(r[:], lnm[:], F.Exp, scale=-0.5)
            return r

        def emit_mean(src16, t, mean_ps, name):
            """accumulate mean(src16^2) over feature tiles into [1, L] psum."""
            sq = spool.tile([128, L], FP16, name=f"sq_{name}_{t}", tag="sq",
                            bufs=2)
            nc.vector.tensor_mul(sq[:], src16[:], src16[:])
            nc.tensor.matmul(mean_ps[:], onesd[:], sq[:],
                             start=(t == 0), stop=(t == 7))

        # ---------- patch embed ----------
        x0 = apool.tile([128, 2, L], FP16, name="x0")
        nc.sync.dma_start(x0[:], d["x0T"].rearrange("(k p) t -> p k t", p=128))
        mean_e = ps_acc.tile([1, L], FP32, name="mean_e", tag="acc")
        for t in range(8):
            wt = wpool.tile([128, PDP], FP16, name=f"pe_w{t}", tag="w")
            nc.sync.dma_start(wt[:], d["pe_t"][t])
            ps = ps_g.tile([128, L], FP32, name=f"pe_ps{t}", tag="g")
            for k in range(2):
                nc.tensor.matmul(ps[:], wt[:, k * 128:(k + 1) * 128],
                                 x0[:, k, :], start=(k == 0), stop=(k == 1))
            nc.scalar.activation(xs[t][:], ps[:], F.Copy)
            nc.vector.tensor_copy(x16[t][:], xs[t][:])
            sqe = spool.tile([128, L], FP16, name=f"sqe_{t}", tag="sq", bufs=2)
            nc.scalar.activation(sqe[:], ps[:], F.Square)
            nc.tensor.matmul(mean_e[:], onesd[:], sqe[:],
                             start=(t == 0), stop=(t == 7))
        re_sb = rms_recip(mean_e[:], "emb")
        mean_in = ps_acc.tile([1, L], FP32, name="mean_l0", tag="acc")
        for t in range(8):
            bc = ps_bc.tile([128, L], FP32, name=f"bc_emb_{t}", tag="bc")
            nc.tensor.matmul(bc[:], escs[0:1, t * 128:(t + 1) * 128], re_sb[:],
                             start=True, stop=True)
            nc.vector.tensor_mul(xs[t][:], xs[t][:], bc[:])
            nc.vector.tensor_copy(x16[t][:], xs[t][:])
            emit_mean(x16[t][:], t, mean_in[:], "l0")

        # ---------- layers ----------
        for l in range(nl):
            mean_in = _layer(nc, tc, d, l, xs, x16, cpool, wpool, apool, spool,
                             ps_g, ps_acc, ps_bc,
                             b16, e1s, ones1, o1c, onesd, ep9, epsb, identf,
                             rms_recip, mean_in, emit_mean, pswc)

        # ---------- final norm + head ----------
        rf_sb = rms_recip(mean_in[:], "fin")
        hN = [apool.tile([128, L], FP16, name=f"hN{t}", tag=f"h2{t}")
              for t in range(8)]
        bcf = ps_bc.tile([128, L], FP32, name="bc_fin", tag="bc")
        nc.tensor.matmul(bcf[:], ones1[:], rf_sb[:], start=True, stop=True)
        bcfh = spool.tile([128, L], FP16, name="bcfh", tag="bchh", bufs=1)
        nc.vector.tensor_copy(bcfh[:], bcf[:])
        for t in range(8):
            nc.vector.tensor_mul(hN[t][:], x16[t][:], bcfh[:])
        for o in range(2):
            wt = wpool.tile([128, D], FP16, name=f"hd_w{o}", tag="w")
            nc.sync.dma_start(wt[:], d["hd_t"][o])
            ps = ps_g.tile([128, L], FP32, name=f"hd_ps{o}", tag="g")
            for k in range(KT):
                nc.tensor.matmul(ps[:], wt[:, k * 128:(k + 1) * 128],
                                 hN[k][:], start=(k == 0), stop=(k == KT - 1))
            rows = 128 if o == 0 else PD - 128
            ot = apool.tile([128, L], FP32, name=f"hd_o{o}")
            nc.scalar.activation(ot[:rows, :], ps[:rows, :], F.Copy)
            nc.sync.dma_start(out_d[o * 128:o * 128 + rows, :], ot[:rows, :])


def _layer(nc, tc, d, l, xs, x16, cpool, wpool, apool, spool,
           ps_g, ps_acc, ps_bc,
           b16, e1s, ones1, o1c, onesd, ep9, epsb, identf, rms_recip,
           mean_in, emit_mean, pswc):
    # per-layer rope/scale tables (prefetched; DMA engine is idle)
    rtab = spool.tile([128, 4, L], FP16, name=f"rtab_{l}", tag="rtab", bufs=2)
    nc.sync.dma_start(rtab[:, 0, :], d["cq"][l])
    nc.sync.dma_start(rtab[:, 1, :], d["sq"][l])
    nc.sync.dma_start(rtab[:, 2, :], d["ck"][l])
    nc.sync.dma_start(rtab[:, 3, :], d["sk"][l])

    # mean1-derived values (mean_in accumulated at the previous layer's tail)
    zs = spool.tile([1, L], FP16, name=f"zs_{l}", tag="zs")
    nc.vector.tensor_scalar_mul(zs[:], mean_in[:], EPS / EP9)
    r1 = rms_recip(mean_in[:], f"r1_{l}", dtype=FP32)
    rT = spool.tile([128, 4], FP32, name=f"rT_{l}", tag="rT")

    # msq accumulators: q at psum partitions 0-15 (bank 1), k at partitions
    # 32-47 (bank 2) so the accumulation matmuls pair up on disjoint PE
    # column groups and run concurrently.
    msq_q = ps_acc.tile([16, L], FP32, name=f"msqq_{l}", tag="acc")
    msq_k = ps_acc.tile([48, L], FP32, name=f"msqk_{l}", tag="acc")

    # ---- Phase A: qkv q/k GEMM + statistics, tensor queue kept dense ----
    qraw = []
    for grp in range(4):
        if grp == 1:
            # transpose r1 -> rT [128, 4] (per-token scale for token-major
            # v); emitted behind the first qkv chains so the tensor queue
            # head never blocks on the scalar r1 chain.
            trp = ps_bc.tile([128, 4], FP32, name=f"trp_{l}", tag="bc")
            for b in range(4):
                nc.tensor.transpose(trp[:, b:b + 1],
                                    r1[:, b * 128:(b + 1) * 128], identf[:])
            nc.vector.tensor_copy(rT[:], trp[:])
        wt = wpool.tile([128, 4, D], FP16, name=f"qkw_{l}_{grp}", tag="w")
        nc.sync.dma_start(wt[:], d["qk_t"][l, grp * 4:(grp + 1) * 4]
                          .rearrange("g p n -> p g n"))
        for gi in range(4):
            ot = grp * 4 + gi
            ps = ps_g.tile([128, L], FP32, name=f"qk_ps_{l}_{ot}", tag="g")
            for k in range(KT):
                nc.tensor.matmul(ps[:], wt[:, gi, k * 128:(k + 1) * 128],
                                 x16[k][:], start=(k == 0), stop=(k == KT - 1))
            qr = spool.tile([128, L], FP16, name=f"qraw_{l}_{ot}", tag="qraw",
                            bufs=16)
            nc.vector.tensor_copy(qr[:], ps[:])
            qraw.append(qr)
            sq = spool.tile([128, L], FP16, name=f"qsq_{l}_{ot}", tag="sq",
                            bufs=2)
            nc.vector.tensor_mul(sq[:], qr[:], qr[:])
            tt = ot % 8
            if ot < 8:
                nc.tensor.matmul(msq_q[:], b16[:, tt * 16:(tt + 1) * 16],
                                 sq[:], start=(tt == 0), stop=False,
                                 tile_position=(0, 0))
            else:
                nc.tensor.matmul(msq_k[32:48, :], b16[:, tt * 16:(tt + 1) * 16],
                                 sq[:], start=(tt == 0), stop=False,
                                 tile_position=(0, 32))

    # eps correction: msq += ep9 * zs  (= eps * ir2); 2-way packed
    nc.tensor.matmul(msq_q[:], ep9[:], zs[:], start=False, stop=True,
                     tile_position=(0, 0))
    nc.tensor.matmul(msq_k[32:48, :], ep9[:], zs[:], start=False, stop=True,
                     tile_position=(0, 32))

    # per-head q/k norm multipliers: alf rows 0-15 = q, rows 32-47 = k
    alf = spool.tile([48, L], FP16, name=f"alf_{l}", tag="alf")
    for row, msq in ((0, msq_q[:]), (32, msq_k[32:48, :])):
        tl = spool.tile([16, L], FP32, name=f"aln_{l}_{row}", tag="a32")
        nc.scalar.activation(tl[:], msq, F.Ln)
        nc.scalar.activation(alf[row:row + 16, :], tl[:], F.Exp, scale=-0.5)

    # ---- Phase B: v GEMM + rope, interleaved ----
    vsb = [apool.tile([128, 16 * 65], FP16, name=f"vsb_{l}_{b}", tag=f"vsb{b}")
           for b in range(4)]
    for b in range(4):
        nc.gpsimd.memset(
            vsb[b][:].rearrange("p (h c) -> p h c", c=65)[:, :, 64:65], 1.0)
    wva = wpool.tile([128, 4, D], FP16, name=f"vwa_{l}", tag="w")
    nc.sync.dma_start(wva[:], d["wv_p"][l, 0:4].rearrange("k p n -> p k n"))
    wvb = wpool.tile([128, 4, D], FP16, name=f"vwb_{l}", tag="w")
    nc.sync.dma_start(wvb[:], d["wv_p"][l, 4:8].rearrange("k p n -> p k n"))

    qf = [None] * 16

    def emit_rope(ot):
        t = ot % 8
        arow = 0 if ot < 8 else 32
        ci, si = (0, 1) if ot < 8 else (2, 3)
        bc = ps_bc.tile([128, L], FP32, name=f"rbc_{l}_{ot}", tag="bc")
        nc.tensor.matmul(bc[:], e1s[arow:arow + 16, t * 128:(t + 1) * 128],
                         alf[arow:arow + 16, :], start=True, stop=True)
        sw = spool.tile([128, L], FP16, name=f"rsw_{l}_{ot}", tag="u2")
        if DBG_NO_SHUF:
            swp = ps_bc.tile([128, L], FP32, name=f"rswp_{l}_{ot}", tag="bc")
            nc.tensor.matmul(swp[:], pswc[:], qraw[ot][:], start=True,
                             stop=True)
            nc.vector.tensor_copy(sw[:], swp[:])
        else:
            nc.vector.stream_shuffle(sw[:], qraw[ot][:], SWAP_MASK)
        u1 = spool.tile([128, L], FP16, name=f"u1_{l}_{ot}", tag="u1")
        nc.vector.tensor_mul(u1[:], qraw[ot][:], rtab[:, ci, :])
        u2 = spool.tile([128, L], FP16, name=f"u2_{l}_{ot}", tag="u1")
        nc.vector.tensor_mul(u2[:], sw[:], rtab[:, si, :])
        nc.vector.tensor_add(u1[:], u1[:], u2[:])
        qt = apool.tile([128, L], FP16, name=f"qf_{l}_{ot}", tag=f"qf{ot}")
        nc.vector.tensor_mul(qt[:], u1[:], bc[:])
        qf[ot] = qt

    def emit_v(b, n):
        ps = ps_g.tile([128, 512], FP32, name=f"v_ps_{l}_{b}_{n}", tag="g")
        for k in range(KT):
            wv = wva if k < 4 else wvb
            nc.tensor.matmul(ps[:], x16[k][:, b * 128:(b + 1) * 128],
                             wv[:, k % 4, n * 512:(n + 1) * 512],
                             start=(k == 0), stop=(k == KT - 1))
        dst = vsb[b][:].rearrange("p (h c) -> p h c", c=65)[:, n * 8:(n + 1) * 8, 0:64]
        nc.vector.tensor_scalar_mul(dst, ps[:], rT[:, b:b + 1])

    # ---- Phase C: attention, software-pipelined with v/rope as filler ----
    oun = [apool.tile([128, L], FP16, name=f"oun_{l}_{t}", tag=f"oun{t}")
           for t in range(8)]
    of = [spool.tile([128, L], FP16, name=f"of_{l}_{t}", tag=f"of{t}", bufs=1)
          for t in range(8)]
    est_store = {}

    def emit_scores(ti):
        tiles = []
        for kt in range(4):
            q0 = 128 * kt
            sta = ps_g.tile([128, L], FP32, name=f"st_{l}_{ti}a_{kt}", tag="g")
            stb = ps_g.tile([128, L], FP32, name=f"st_{l}_{ti}b_{kt}", tag="g")
            nc.tensor.matmul(sta[:, q0:], qf[8 + ti][0:64, kt * 128:(kt + 1) * 128],
                             qf[ti][0:64, q0:], start=True, stop=True)
            nc.tensor.matmul(stb[:, q0:], qf[8 + ti][64:128, kt * 128:(kt + 1) * 128],
                             qf[ti][64:128, q0:], start=True, stop=True)
            esta = spool.tile([128, L], FP16, name=f"est_{l}_{ti}a_{kt}",
                              tag="esta", bufs=12)
            estb = spool.tile([128, L], FP16, name=f"est_{l}_{ti}b_{kt}",
                              tag="estb", bufs=12)
            nc.scalar.activation(esta[:, q0:], sta[:, q0:], F.Exp, scale=0.125)
            nc.scalar.activation(estb[:, q0:], stb[:, q0:], F.Exp, scale=0.125)
            nc.gpsimd.memset(esta[64:128, q0:q0 + 64], 0.0)
            nc.gpsimd.memset(estb[64:128, q0:q0 + 64], 0.0)
            tiles.append((esta, estb))
        est_store[ti] = tiles

    def emit_av(ti):
        ha, hb = 2 * ti, 2 * ti + 1
        oea = ps_acc.tile([65, L], FP32, name=f"oe_{l}_{ha}", tag="acc")
        oeb = ps_acc.tile([65, L], FP32, name=f"oe_{l}_{hb}", tag="acc")
        tiles = est_store.pop(ti)
        for kt in range(4):
            q0 = 128 * kt
            esta, estb = tiles[kt]
            nc.tensor.matmul(oea[:, q0:], vsb[kt][:, ha * 65:(ha + 1) * 65],
                             esta[:, q0:], start=(kt == 0), stop=(kt == 3))
            nc.tensor.matmul(oeb[:, q0:], vsb[kt][:, hb * 65:(hb + 1) * 65],
                             estb[:, q0:], start=(kt == 0), stop=(kt == 3))
        # denominator reciprocal via exp(-ln(den)) on [33, L] staging (both
        # functions live in the resident natlog_exp table set)
        den = spool.tile([33, L], FP32, name=f"den_{l}_{ti}", tag="den",
                         bufs=2)
        nc.gpsimd.memset(den[:], 1.0)
        nc.vector.tensor_copy(den[0:1, :], oea[64:65, :])
        nc.vector.tensor_copy(den[32:33, :], oeb[64:65, :])
        lnd = spool.tile([33, L], FP32, name=f"lnd_{l}_{ti}", tag="lnd",
                         bufs=2)
        nc.scalar.activation(lnd[:], den[:], F.Ln)
        rden = spool.tile([33, L], FP16, name=f"rden_{l}_{ti}", tag="rden",
                          bufs=2)
        nc.scalar.activation(rden[:], lnd[:], F.Exp, scale=-1.0)
        nc.vector.tensor_copy(oun[ti][0:64, :], oea[0:64, :])
        nc.vector.tensor_copy(oun[ti][64:128, :], oeb[0:64, :])
        # broadcast 1/den over partitions; 2-way packed (rows 0-63 / 64-127)
        bc = ps_bc.tile([128, L], FP32, name=f"nbc_{l}_{ti}", tag="bc")
        nc.tensor.matmul(bc[0:64, :], o1c[0:1, :], rden[0:1, :],
                         start=True, stop=True, tile_position=(0, 0))
        nc.tensor.matmul(bc[64:128, :], o1c[32:33, :], rden[32:33, :],
                         start=True, stop=True, tile_position=(32, 64))
        nc.vector.tensor_mul(of[ti][:], oun[ti][:], bc[:])

    # v + rope first (dense filler while alpha/stats chains settle), then
    # the score/attn-v pipeline with a 2-stage exp lead.
    rope_order = [x for p in zip(range(8), range(8, 16)) for x in p]
    vchunks = [(b, n) for n in range(2) for b in range(4)]
    for i in range(8):
        emit_rope(rope_order[2 * i])
        emit_rope(rope_order[2 * i + 1])
        emit_v(*vchunks[i])
    emit_scores(0)
    emit_scores(1)
    emit_scores(2)
    for ti in range(3, 8):
        emit_av(ti - 3)
        emit_scores(ti)
    emit_av(5)
    emit_av(6)
    emit_av(7)

    # ---- Phase D: out projection + residual + mean2 ----
    mean2 = ps_acc.tile([1, L], FP32, name=f"mean2_{l}", tag="acc")
    for grp in range(2):
        wt = wpool.tile([128, 4, D], FP16, name=f"wo_{l}_{grp}", tag="w")
        nc.sync.dma_start(wt[:], d["wo_t"][l, grp * 4:(grp + 1) * 4]
                          .rearrange("g p n -> p g n"))
        for gi in range(4):
            t = grp * 4 + gi
            ps = ps_g.tile([128, L], FP32, name=f"xa_ps_{l}_{t}", tag="g")
            for k in range(KT):
                nc.tensor.matmul(ps[:], wt[:, gi, k * 128:(k + 1) * 128],
                                 of[k][:], start=(k == 0), stop=(k == KT - 1))
            nc.vector.scalar_tensor_tensor(x16[t][:], ps[:], 0.0, xs[t][:],
                                           ALU.add, ALU.add)
            nc.vector.tensor_add(xs[t][:], xs[t][:], ps[:])
            emit_mean(x16[t][:], t, mean2[:], f"m2_{l}")

    # ---- Phase E: MLP ----
    r2 = rms_recip(mean2[:], f"r2_{l}")
    h2 = [apool.tile([128, L], FP16, name=f"h2_{l}_{t}", tag=f"h2{t}")
          for t in range(8)]
    bch = ps_bc.tile([128, L], FP32, name=f"bch_{l}", tag="bc")
    nc.tensor.matmul(bch[:], ones1[:], r2[:], start=True, stop=True)
    bchh = spool.tile([128, L], FP16, name=f"bchh_{l}", tag="bchh", bufs=1)
    nc.vector.tensor_copy(bchh[:], bch[:])
    for t in range(8):
        nc.vector.tensor_mul(h2[t][:], x16[t][:], bchh[:])

    pj = []
    for j in range(JT):
        wt = wpool.tile([128, 4, D], FP16, name=f"gu_{l}_{j}", tag="w")
        nc.sync.dma_start(wt[:], d["gu_t"][l, j * 4:(j + 1) * 4]
                          .rearrange("g p n -> p g n"))
        # consume each psum right after its group (ps_g rotates 2 slots)
        sg1 = spool.tile([128, L], FP16, name=f"sg1_{l}_{j}", tag="sg1")
        sg2 = spool.tile([128, L], FP16, name=f"sg2_{l}_{j}", tag="sg2")
        ta = spool.tile([128, L], FP16, name=f"ta_{l}_{j}", tag="ta")
        tb = spool.tile([128, L], FP16, name=f"tb_{l}_{j}", tag="tb")
        for gi in range(4):
            ps = ps_g.tile([128, L], FP32, name=f"gu_ps_{l}_{j}_{gi}", tag="g")
            for k in range(KT):
                nc.tensor.matmul(ps[:], wt[:, gi, k * 128:(k + 1) * 128],
                                 h2[k][:], start=(k == 0), stop=(k == KT - 1))
            if gi == 0:
                nc.scalar.activation(sg1[:], ps[:], F.Sigmoid if SIM_SAFE else F.Silu)
            elif gi == 1:
                nc.scalar.activation(sg2[:], ps[:], F.Sigmoid if SIM_SAFE else F.Silu)
            elif gi == 2:
                nc.vector.scalar_tensor_tensor(ta[:], ps[:], 0.0, sg1[:],
                                               ALU.add, ALU.mult)
            else:
                nc.vector.scalar_tensor_tensor(tb[:], ps[:], 0.0, sg2[:],
                                               ALU.add, ALU.mult)
        p = spool.tile([128, L], FP16, name=f"p_{l}_{j}", tag=f"p{j}", bufs=1)
        nc.vector.tensor_add(p[:], ta[:], tb[:])
        pj.append(p)

    mean_next = ps_acc.tile([1, L], FP32, name=f"mean_{l + 1}", tag="acc")
    for grp in range(4):
        wt = wpool.tile([128, 2, IHP], FP16, name=f"dn_{l}_{grp}", tag="w")
        nc.sync.dma_start(wt[:], d["dn_t"][l, grp * 2:(grp + 1) * 2]
                          .rearrange("g p n -> p g n"))
        for gi in range(2):
            t = grp * 2 + gi
            ps = ps_g.tile([128, L], FP32, name=f"dn_ps_{l}_{t}", tag="g")
            for j in range(JT):
                nc.tensor.matmul(ps[:], wt[:, gi, j * 128:(j + 1) * 128],
                                 pj[j][:], start=(j == 0), stop=(j == JT - 1))
            nc.vector.scalar_tensor_tensor(x16[t][:], ps[:], 0.0, xs[t][:],
                                           ALU.add, ALU.add)
            nc.vector.tensor_add(xs[t][:], xs[t][:], ps[:])
            emit_mean(x16[t][:], t, mean_next[:], f"mn_{l}")
    return mean_next


# ----------------------------------------------------------------------
# entry point
# ----------------------------------------------------------------------

def _get_nc(nl=NL):
    if nl not in _CACHE:
        _CACHE[nl] = _build(nl)
    return _CACHE[nl]


def run(inputs, nl=NL, trace=False):
    inputs = {k: np.asarray(v) for k, v in inputs.items()}
    w = _prep_weights(inputs)
    in_maps = []
    for b in range(N_CORES):
        tok = _patchify(inputs["frames"][b]).astype(np.float32)
        x0T = np.zeros((PDP, L), np.float16)
        x0T[:PD] = tok.T.astype(np.float16)
        m = {"x0T": x0T, "qk_t": w["qk_t"], "wv_p": w["wv_p"],
             "wo_t": w["wo_t"], "gu_t": w["gu_t"], "dn_t": w["dn_t"],
             "pe_t": w["pe_t"], "hd_t": w["hd_t"],
             "e1": w["e1"], "esc": w["esc"],
             "cq": w["cq"], "sq": w["sq"], "ck": w["ck"], "sk": w["sk"],
             "b16": w["b16"], "psw": w["psw"]}
        in_maps.append(m)
    nc = _get_nc(nl)
    res = run_bass_kernel_spmd(nc, in_maps, list(range(N_CORES)), trace=trace)
    outs = []
    for b in range(N_CORES):
        tok = res.results[b]["out"].T  # (L, PD)
        outs.append(_unpatchify(tok))
    return np.stack(outs).astype(np.float32), res


def kernel(**inputs) -> np.ndarray:
    out, _ = run(inputs)
    return out



# revision 30
# speedup vs baseline: 1.0148x; 1.0148x over previous
"""AR video patch transformer forward on 8 Trainium2 NeuronCores.

Strategy: pure data parallelism — each core runs the full 8-layer
transformer on one batch element. Host does patchify/unpatchify and
weight preprocessing (scale folds, padding, lhsT tiling, fp16 cast).

v2 vs v1:
 - Emission reordered so the tensor queue stays dense (HAM clock-gate
   stays at 8/8 instead of oscillating to half clock in attention).
 - Attention softmax denominators: DVE reciprocal_approx_fast instead of
   a scalar Ln -> table swap -> Exp chain (removes a ~6.4us/layer stall).
 - RoPE pair-swap via DVE stream_shuffle (was a PE matmul).
 - All scalar rsqrt via Ln+Exp; a post-compile pass rewrites activation
   table-set ids to the combined natural_log_exp set and drops redundant
   loads (74 -> ~18 table loads).
 - Aux matmuls 2-way packed on the PE array (msq q/k, rope alpha bcast).
 - Fused psum-consume ops on DVE (scalar_tensor_tensor) for residual
   adds and gate*up products.

v3 vs v2:
 - MLP gate/up psums consumed immediately after each accumulation group
   (safe under a 2-slot psum rotation; keeps the PE queue dense).
 - r2/rf broadcast staged to fp16 once, so the 8 h2/hN row scalings run
   at 2x DVE rate instead of re-reading the fp32 psum broadcast.
 - Known-dead-end notes: fusing the per-kt softmax exps via packed score
   psums ([128,1024] 2-bank or single-bank layouts) compiles + passes
   CoreSim but faults at runtime on HW; GpSimd cannot read PSUM (BIR
   verifier); DVE reciprocal custom ops require SBUF fp32 in/out;
   batching the den Ln/Exp across ti-pairs regresses ~190us (stalls the
   attn-v pipeline).  fp8 is out of error budget (absmax/rms gate 2e-2,
   fp16 sits at 4e-3, one e4m3 GEMM costs ~3%).
"""

import numpy as np

import concourse.bass as bass
import concourse.mybir as mybir
from concourse import bacc
from concourse.tile import TileContext
from concourse.bass_utils import run_bass_kernel_spmd

F = mybir.ActivationFunctionType
ALU = mybir.AluOpType
FP16 = mybir.dt.float16
FP32 = mybir.dt.float32

# Model config (hardcoded from the problem spec)
B = 8; T = 8; C = 3; RES = 64; P = 8
D = 1024; NH = 16; HD = 64; NL = 8
INNER = 2730
NP_ = 64           # patches per frame
PD = 192           # patch dim
PDP = 256          # padded patch dim (2 k-tiles)
L = 512            # tokens
EPS = 1e-6
KT = D // 128      # 8
IH = INNER // 2    # 1365 half-inner
IHP = 1408         # padded half-inner (11 tiles)
JT = IHP // 128    # 11
EP9 = 2.0 ** -9    # exact fp16 scalar used for the eps matmul

N_CORES = 8
_CACHE = {}

import os
DBG_NO_TBLFIX = os.environ.get("K_NO_TBLFIX", "") == "1"
DBG_NO_SHUF = os.environ.get("K_NO_SHUF", "") == "1"
DBG_NO_RECIP = os.environ.get("K_NO_RECIP", "") == "1"
# CoreSim has no Silu; K_SIMSAFE swaps in Sigmoid (structure-identical)
SIM_SAFE = os.environ.get("K_SIMSAFE", "") == "1"

SWAP_MASK = [i ^ 1 for i in range(32)]

# activation table sets (trn2/cayman act_info.json order)
SET_NATLOG_EXP = 6     # ln, exp, square, copy, ...
_REWRITE_SETS = {0, 5}  # exp_and_others / natural_log -> natlog_exp


class _Bacc(bacc.Bacc):
    """Bacc with a post-pass that merges ln/exp table sets and deletes
    redundant table loads (the stock pass picks the first set containing
    each function, so ln<->exp sequences thrash)."""

    def insert_act_table_loads(self):
        super().insert_act_table_loads()
        if DBG_NO_TBLFIX:
            return
        for blk in self.main_func.blocks:
            resident = None
            keep = []
            for inst in blk.instructions:
                if isinstance(inst, mybir.InstLoadActFuncSet):
                    if inst.act_func_set_id in _REWRITE_SETS:
                        inst.act_func_set_id = SET_NATLOG_EXP
                    if inst.act_func_set_id == resident and not inst.sync_info:
                        continue  # redundant, drop
                    resident = inst.act_func_set_id
                keep.append(inst)
            blk.instructions[:] = keep


# ----------------------------------------------------------------------
# host-side preprocessing
# ----------------------------------------------------------------------

def _lhsT_tile(w):
    """[Din, Dout] -> [Dout/128, 128, Din] fp16 lhsT-tiled blocks."""
    din, dout = w.shape
    kt, ot = din // 128, dout // 128
    return np.ascontiguousarray(
        w.reshape(kt, 128, ot, 128).transpose(2, 1, 0, 3).reshape(ot, 128, din)
    ).astype(np.float16)


def _rope_tables(scale):
    """C/S tables [128, L] with the per-dim norm scale folded in
    (scale applied before rotation, matching the reference order)."""
    q = HD // 4  # 16
    inv = 1.0 / (10000.0 ** (np.arange(q, dtype=np.float64) / q))
    t_idx = np.repeat(np.arange(T), NP_)
    s_idx = np.tile(np.arange(NP_), T)
    ang = np.concatenate(
        [t_idx[:, None] * inv[None, :], s_idx[:, None] * inv[None, :]], axis=1
    )  # (L, 32)
    cdm = np.zeros((128, L), np.float64)
    sdm = np.zeros((128, L), np.float64)
    for d in range(128):
        dl = d % 64
        i = dl // 2
        cdm[d] = np.cos(ang[:, i]) * scale[dl]
        sg = -1.0 if d % 2 == 0 else 1.0
        sdm[d] = sg * np.sin(ang[:, i]) * scale[dl ^ 1]
    return cdm.astype(np.float16), sdm.astype(np.float16)


def _prep_weights(inp):
    w = {}
    n1 = inp["norm1_scale"]; n2 = inp["norm2_scale"]
    qk_t = np.empty((NL, 16, 128, D), np.float16)
    wv_p = np.empty((NL, KT, 128, D), np.float16)
    wo_t = np.empty((NL, 8, 128, D), np.float16)
    gu_t = np.empty((NL, 44, 128, D), np.float16)
    dn_t = np.empty((NL, 8, 128, IHP), np.float16)
    for l in range(NL):
        w1 = inp["qkv_w"][l] * n1[l][:, None]
        qk_t[l] = _lhsT_tile(w1[:, :2048])
        wv_p[l] = w1[:, 2048:].reshape(KT, 128, D).astype(np.float16)
        wo_t[l] = _lhsT_tile(inp["out_w"][l])
        g = inp["gate_w"][l] * n2[l][:, None]
        u = inp["up_w"][l] * n2[l][:, None]
        gp = np.zeros((D, 2 * IHP), np.float32)
        up = np.zeros((D, 2 * IHP), np.float32)
        gp[:, :IH] = g[:, :IH]; gp[:, IHP:IHP + IH] = g[:, IH:]
        up[:, :IH] = u[:, :IH]; up[:, IHP:IHP + IH] = u[:, IH:]
        gt = _lhsT_tile(gp); ut = _lhsT_tile(up)
        order = []
        for j in range(JT):
            order += [gt[j], gt[JT + j], ut[j], ut[JT + j]]
        gu_t[l] = np.stack(order)
        dp = np.zeros((IHP, D), np.float32)
        dp[:IH] = inp["down_w"][l]
        dn_t[l] = _lhsT_tile(dp)
    w["qk_t"] = qk_t; w["wv_p"] = wv_p; w["wo_t"] = wo_t
    w["gu_t"] = gu_t; w["dn_t"] = dn_t

    pe = np.zeros((PDP, D), np.float32)
    pe[:PD] = inp["patch_embed_w"]
    w["pe_t"] = _lhsT_tile(pe)
    hw = np.zeros((D, PDP), np.float32)
    hw[:, :PD] = inp["head_w"] * inp["normf_scale"][:, None]
    w["hd_t"] = _lhsT_tile(hw)

    # per-head broadcast matrix (block ones); q rows 0-15, k rows 32-47
    e1 = np.zeros((48, 8, 128), np.float16)
    for t in range(8):
        for dl in range(128):
            e1[2 * t + dl // 64, t, dl] = 1.0
            e1[32 + 2 * t + dl // 64, t, dl] = 1.0
    w["e1"] = np.ascontiguousarray(e1.reshape(48, 8 * 128))

    w["esc"] = inp["embed_norm_scale"].reshape(1, 8 * 128).astype(np.float16)

    cq = np.empty((NL, 128, L), np.float16); sq = np.empty((NL, 128, L), np.float16)
    ck = np.empty((NL, 128, L), np.float16); sk = np.empty((NL, 128, L), np.float16)
    for l in range(NL):
        cq[l], sq[l] = _rope_tables(inp["q_norm_scale"][l])
        ck[l], sk[l] = _rope_tables(inp["k_norm_scale"][l])
    w["cq"] = cq; w["sq"] = sq; w["ck"] = ck; w["sk"] = sk
    b16 = np.zeros((128, 8 * 16), np.float16)
    for t in range(8):
        for dl in range(128):
            b16[dl, t * 16 + 2 * t + dl // 64] = 1.0 / 64.0
    w["b16"] = b16
    psw = np.zeros((128, 128), np.float16)
    for i in range(128):
        psw[i ^ 1, i] = 1.0
    w["psw"] = psw
    return w


def _patchify(frames_b):
    # (T, C, RES, RES) -> (L, PD)
    h = RES // P
    x = frames_b.reshape(T, C, h, P, h, P)
    x = x.transpose(0, 2, 4, 1, 3, 5).reshape(T * h * h, C * P * P)
    return x


def _unpatchify(tokens):
    # (L, PD) -> (T, C, RES, RES)
    h = RES // P
    y = tokens.reshape(T, h, h, C, P, P)
    return y.transpose(0, 3, 1, 4, 2, 5).reshape(T, C, RES, RES)


# ----------------------------------------------------------------------
# device kernel
# ----------------------------------------------------------------------

def _build(nl=NL):
    nc = _Bacc()
    d = {}
    d["x0T"] = nc.dram_tensor("x0T", [PDP, L], FP16, kind="ExternalInput")
    d["qk_t"] = nc.dram_tensor("qk_t", [NL, 16, 128, D], FP16, kind="ExternalInput")
    d["wv_p"] = nc.dram_tensor("wv_p", [NL, KT, 128, D], FP16, kind="ExternalInput")
    d["wo_t"] = nc.dram_tensor("wo_t", [NL, 8, 128, D], FP16, kind="ExternalInput")
    d["gu_t"] = nc.dram_tensor("gu_t", [NL, 44, 128, D], FP16, kind="ExternalInput")
    d["dn_t"] = nc.dram_tensor("dn_t", [NL, 8, 128, IHP], FP16, kind="ExternalInput")
    d["pe_t"] = nc.dram_tensor("pe_t", [8, 128, PDP], FP16, kind="ExternalInput")
    d["hd_t"] = nc.dram_tensor("hd_t", [2, 128, D], FP16, kind="ExternalInput")
    d["e1"] = nc.dram_tensor("e1", [48, 8 * 128], FP16, kind="ExternalInput")
    d["esc"] = nc.dram_tensor("esc", [1, 8 * 128], FP16, kind="ExternalInput")
    d["cq"] = nc.dram_tensor("cq", [NL, 128, L], FP16, kind="ExternalInput")
    d["sq"] = nc.dram_tensor("sq", [NL, 128, L], FP16, kind="ExternalInput")
    d["ck"] = nc.dram_tensor("ck", [NL, 128, L], FP16, kind="ExternalInput")
    d["sk"] = nc.dram_tensor("sk", [NL, 128, L], FP16, kind="ExternalInput")
    d["b16"] = nc.dram_tensor("b16", [128, 8 * 16], FP16, kind="ExternalInput")
    d["psw"] = nc.dram_tensor("psw", [128, 128], FP16, kind="ExternalInput")
    out_d = nc.dram_tensor("out", [PD, L], FP32, kind="ExternalOutput")

    with TileContext(nc) as tc:
        _emit(nc, tc, d, out_d, nl)
    nc.compile()
    return nc


def _emit(nc, tc, d, out_d, nl):
    import contextlib
    ctx = contextlib.ExitStack()
    with ctx:
        cpool = ctx.enter_context(tc.tile_pool(name="consts", bufs=1))
        xpool = ctx.enter_context(tc.tile_pool(name="x", bufs=1))
        wpool = ctx.enter_context(tc.tile_pool(name="w", bufs=3))
        apool = ctx.enter_context(tc.tile_pool(name="act", bufs=1))
        spool = ctx.enter_context(tc.tile_pool(name="small", bufs=2))
        # psum: tag "g" slots are 2 banks wide ([128,1024] fp32) so the
        # attention scores for one head-pair half (kt0+kt1+kt3 packed) fit
        # one slot; general [128,512] psums use half a slot.  2(g)*2 + 2(acc)
        # + 2(bc) = 8 banks exactly.
        ps_g = ctx.enter_context(tc.tile_pool(name="psg", bufs=4, space="PSUM"))
        ps_acc = ctx.enter_context(tc.tile_pool(name="psacc", bufs=2, space="PSUM"))
        ps_bc = ctx.enter_context(tc.tile_pool(name="psbc", bufs=2, space="PSUM"))

        # ---- persistent constants ----
        b16 = cpool.tile([128, 128], FP16, name="b16")
        nc.sync.dma_start(b16[:], d["b16"][:])
        pswc = cpool.tile([128, 128], FP16, name="pswc")
        nc.sync.dma_start(pswc[:], d["psw"][:])
        e1s = cpool.tile([48, 8 * 128], FP16, name="e1s")
        nc.sync.dma_start(e1s[:], d["e1"][:])
        escs = cpool.tile([1, 8 * 128], FP16, name="escs")
        nc.sync.dma_start(escs[:], d["esc"][:])
        ones1 = cpool.tile([1, 128], FP16, name="ones1")
        nc.gpsimd.memset(ones1[:], 1.0)
        o1c = cpool.tile([33, 64], FP16, name="o1c")
        nc.gpsimd.memset(o1c[:], 1.0)
        onesd = cpool.tile([128, 1], FP16, name="onesd")
        nc.gpsimd.memset(onesd[:], 1.0 / 1024.0)
        ep9 = cpool.tile([1, 16], FP16, name="ep9")
        nc.gpsimd.memset(ep9[:], EP9)
        epsb = cpool.tile([128, 1], FP32, name="epsb")
        nc.gpsimd.memset(epsb[:], EPS)
        identf = cpool.tile([1, 1], FP32, name="identf")
        nc.gpsimd.memset(identf[:], 1.0)

        # residual stream
        xs = [xpool.tile([128, L], FP32, name=f"x{t}") for t in range(8)]
        x16 = [xpool.tile([128, L], FP16, name=f"m{t}") for t in range(8)]

        def rms_recip(mean_ps, name, dtype=FP16):
            """[1, L] psum mean -> [1, L] rsqrt(mean+eps) via Ln+Exp."""
            lnm = spool.tile([1, L], FP32, name=f"lnm_{name}", tag="r32")
            nc.scalar.activation(lnm[:], mean_ps[:], F.Ln, bias=epsb[0:1])
            r = spool.tile([1, L], dtype, name=f"r_{name}", tag="r16")
            nc.scalar.activation(r[:], lnm[:], F.Exp, scale=-0.5)
            return r

        def emit_mean(src16, t, mean_ps, name):
            """accumulate mean(src16^2) over feature tiles into [1, L] psum."""
            sq = spool.tile([128, L], FP16, name=f"sq_{name}_{t}", tag="sq",
                            bufs=2)
            nc.vector.tensor_mul(sq[:], src16[:], src16[:])
            nc.tensor.matmul(mean_ps[:], onesd[:], sq[:],
                             start=(t == 0), stop=(t == 7))

        # ---------- patch embed ----------
        x0 = apool.tile([128, 2, L], FP16, name="x0")
        nc.sync.dma_start(x0[:], d["x0T"].rearrange("(k p) t -> p k t", p=128))
        mean_e = ps_acc.tile([1, L], FP32, name="mean_e", tag="acc")
        for t in range(8):
            wt = wpool.tile([128, PDP], FP16, name=f"pe_w{t}", tag="w")
            nc.sync.dma_start(wt[:], d["pe_t"][t])
            ps = ps_g.tile([128, L], FP32, name=f"pe_ps{t}", tag="g")
            for k in range(2):
                nc.tensor.matmul(ps[:], wt[:, k * 128:(k + 1) * 128],
                                 x0[:, k, :], start=(k == 0), stop=(k == 1))
            nc.scalar.activation(xs[t][:], ps[:], F.Copy)
            nc.vector.tensor_copy(x16[t][:], xs[t][:])
            sqe = spool.tile([128, L], FP16, name=f"sqe_{t}", tag="sq", bufs=2)
            nc.scalar.activation(sqe[:], ps[:], F.Square)
            nc.tensor.matmul(mean_e[:], onesd[:], sqe[:],
                             start=(t == 0), stop=(t == 7))
        re_sb = rms_recip(mean_e[:], "emb")
        mean_in = ps_acc.tile([1, L], FP32, name="mean_l0", tag="acc")
        for t in range(8):
            bc = ps_bc.tile([128, L], FP32, name=f"bc_emb_{t}", tag="bc")
            nc.tensor.matmul(bc[:], escs[0:1, t * 128:(t + 1) * 128], re_sb[:],
                             start=True, stop=True)
            nc.vector.tensor_mul(xs[t][:], xs[t][:], bc[:])
            nc.vector.tensor_copy(x16[t][:], xs[t][:])
            emit_mean(x16[t][:], t, mean_in[:], "l0")

        # ---------- layers ----------
        for l in range(nl):
            mean_in = _layer(nc, tc, d, l, xs, x16, cpool, wpool, apool, spool,
                             ps_g, ps_acc, ps_bc,
                             b16, e1s, ones1, o1c, onesd, ep9, epsb, identf,
                             rms_recip, mean_in, emit_mean, pswc)

        # ---------- final norm + head ----------
        rf_sb = rms_recip(mean_in[:], "fin")
        hN = [apool.tile([128, L], FP16, name=f"hN{t}", tag=f"h2{t}")
              for t in range(8)]
        bcf = ps_bc.tile([128, L], FP32, name="bc_fin", tag="bc")
        nc.tensor.matmul(bcf[:], ones1[:], rf_sb[:], start=True, stop=True)
        bcfh = spool.tile([128, L], FP16, name="bcfh", tag="bchh", bufs=1)
        nc.vector.tensor_copy(bcfh[:], bcf[:])
        for t in range(8):
            nc.vector.tensor_mul(hN[t][:], x16[t][:], bcfh[:])
        for o in range(2):
            wt = wpool.tile([128, D], FP16, name=f"hd_w{o}", tag="w")
            nc.sync.dma_start(wt[:], d["hd_t"][o])
            ps = ps_g.tile([128, L], FP32, name=f"hd_ps{o}", tag="g")
            for k in range(KT):
                nc.tensor.matmul(ps[:], wt[:, k * 128:(k + 1) * 128],
                                 hN[k][:], start=(k == 0), stop=(k == KT - 1))
            rows = 128 if o == 0 else PD - 128
            ot = apool.tile([128, L], FP32, name=f"hd_o{o}")
            nc.scalar.activation(ot[:rows, :], ps[:rows, :], F.Copy)
            nc.sync.dma_start(out_d[o * 128:o * 128 + rows, :], ot[:rows, :])


def _layer(nc, tc, d, l, xs, x16, cpool, wpool, apool, spool,
           ps_g, ps_acc, ps_bc,
           b16, e1s, ones1, o1c, onesd, ep9, epsb, identf, rms_recip,
           mean_in, emit_mean, pswc):
    # per-layer rope/scale tables (prefetched; DMA engine is idle)
    rtab = spool.tile([128, 4, L], FP16, name=f"rtab_{l}", tag="rtab", bufs=2)
    nc.sync.dma_start(rtab[:, 0, :], d["cq"][l])
    nc.sync.dma_start(rtab[:, 1, :], d["sq"][l])
    nc.sync.dma_start(rtab[:, 2, :], d["ck"][l])
    nc.sync.dma_start(rtab[:, 3, :], d["sk"][l])

    # mean1-derived values (mean_in accumulated at the previous layer's tail)
    zs = spool.tile([1, L], FP16, name=f"zs_{l}", tag="zs")
    nc.vector.tensor_scalar_mul(zs[:], mean_in[:], EPS / EP9)
    r1 = rms_recip(mean_in[:], f"r1_{l}", dtype=FP32)
    rT = spool.tile([128, 4], FP32, name=f"rT_{l}", tag="rT")

    # msq accumulators: q at psum partitions 0-15 (bank 1), k at partitions
    # 32-47 (bank 2) so the accumulation matmuls pair up on disjoint PE
    # column groups and run concurrently.
    msq_q = ps_acc.tile([16, L], FP32, name=f"msqq_{l}", tag="acc")
    msq_k = ps_acc.tile([48, L], FP32, name=f"msqk_{l}", tag="acc")

    # ---- Phase A: qkv q/k GEMM + statistics, tensor queue kept dense ----
    qraw = []
    for grp in range(4):
        if grp == 1:
            # transpose r1 -> rT [128, 4] (per-token scale for token-major
            # v); emitted behind the first qkv chains so the tensor queue
            # head never blocks on the scalar r1 chain.
            trp = ps_bc.tile([128, 4], FP32, name=f"trp_{l}", tag="bc")
            for b in range(4):
                nc.tensor.transpose(trp[:, b:b + 1],
                                    r1[:, b * 128:(b + 1) * 128], identf[:])
            nc.vector.tensor_copy(rT[:], trp[:])
        wt = wpool.tile([128, 4, D], FP16, name=f"qkw_{l}_{grp}", tag="w")
        nc.sync.dma_start(wt[:], d["qk_t"][l, grp * 4:(grp + 1) * 4]
                          .rearrange("g p n -> p g n"))
        for gi in range(4):
            ot = grp * 4 + gi
            ps = ps_g.tile([128, L], FP32, name=f"qk_ps_{l}_{ot}", tag="g")
            for k in range(KT):
                nc.tensor.matmul(ps[:], wt[:, gi, k * 128:(k + 1) * 128],
                                 x16[k][:], start=(k == 0), stop=(k == KT - 1))
            qr = spool.tile([128, L], FP16, name=f"qraw_{l}_{ot}", tag="qraw",
                            bufs=16)
            nc.vector.tensor_copy(qr[:], ps[:])
            qraw.append(qr)
            sq = spool.tile([128, L], FP16, name=f"qsq_{l}_{ot}", tag="sq",
                            bufs=2)
            nc.vector.tensor_mul(sq[:], qr[:], qr[:])
            tt = ot % 8
            if ot < 8:
                nc.tensor.matmul(msq_q[:], b16[:, tt * 16:(tt + 1) * 16],
                                 sq[:], start=(tt == 0), stop=False,
                                 tile_position=(0, 0))
            else:
                nc.tensor.matmul(msq_k[32:48, :], b16[:, tt * 16:(tt + 1) * 16],
                                 sq[:], start=(tt == 0), stop=False,
                                 tile_position=(0, 32))

    # eps correction: msq += ep9 * zs  (= eps * ir2); 2-way packed
    nc.tensor.matmul(msq_q[:], ep9[:], zs[:], start=False, stop=True,
                     tile_position=(0, 0))
    nc.tensor.matmul(msq_k[32:48, :], ep9[:], zs[:], start=False, stop=True,
                     tile_position=(0, 32))

    # per-head q/k norm multipliers: alf rows 0-15 = q, rows 32-47 = k
    alf = spool.tile([48, L], FP16, name=f"alf_{l}", tag="alf")
    for row, msq in ((0, msq_q[:]), (32, msq_k[32:48, :])):
        tl = spool.tile([16, L], FP32, name=f"aln_{l}_{row}", tag="a32")
        nc.scalar.activation(tl[:], msq, F.Ln)
        nc.scalar.activation(alf[row:row + 16, :], tl[:], F.Exp, scale=-0.5)

    # ---- Phase B: v GEMM + rope, interleaved ----
    vsb = [apool.tile([128, 16 * 65], FP16, name=f"vsb_{l}_{b}", tag=f"vsb{b}")
           for b in range(4)]
    for b in range(4):
        nc.gpsimd.memset(
            vsb[b][:].rearrange("p (h c) -> p h c", c=65)[:, :, 64:65], 1.0)
    wva = wpool.tile([128, 4, D], FP16, name=f"vwa_{l}", tag="w")
    nc.sync.dma_start(wva[:], d["wv_p"][l, 0:4].rearrange("k p n -> p k n"))
    wvb = wpool.tile([128, 4, D], FP16, name=f"vwb_{l}", tag="w")
    nc.sync.dma_start(wvb[:], d["wv_p"][l, 4:8].rearrange("k p n -> p k n"))

    qf = [None] * 16

    def emit_rope(ot):
        t = ot % 8
        arow = 0 if ot < 8 else 32
        ci, si = (0, 1) if ot < 8 else (2, 3)
        bc = ps_bc.tile([128, L], FP32, name=f"rbc_{l}_{ot}", tag="bc")
        nc.tensor.matmul(bc[:], e1s[arow:arow + 16, t * 128:(t + 1) * 128],
                         alf[arow:arow + 16, :], start=True, stop=True)
        sw = spool.tile([128, L], FP16, name=f"rsw_{l}_{ot}", tag="u2")
        if DBG_NO_SHUF:
            swp = ps_bc.tile([128, L], FP32, name=f"rswp_{l}_{ot}", tag="bc")
            nc.tensor.matmul(swp[:], pswc[:], qraw[ot][:], start=True,
                             stop=True)
            nc.vector.tensor_copy(sw[:], swp[:])
        else:
            nc.vector.stream_shuffle(sw[:], qraw[ot][:], SWAP_MASK)
        u1 = spool.tile([128, L], FP16, name=f"u1_{l}_{ot}", tag="u1")
        nc.vector.tensor_mul(u1[:], qraw[ot][:], rtab[:, ci, :])
        u2 = spool.tile([128, L], FP16, name=f"u2_{l}_{ot}", tag="u1")
        nc.vector.tensor_mul(u2[:], sw[:], rtab[:, si, :])
        nc.vector.tensor_add(u1[:], u1[:], u2[:])
        qt = apool.tile([128, L], FP16, name=f"qf_{l}_{ot}", tag=f"qf{ot}")
        nc.vector.tensor_mul(qt[:], u1[:], bc[:])
        qf[ot] = qt

    def emit_v(b, n):
        ps = ps_g.tile([128, 512], FP32, name=f"v_ps_{l}_{b}_{n}", tag="g")
        for k in range(KT):
            wv = wva if k < 4 else wvb
            nc.tensor.matmul(ps[:], x16[k][:, b * 128:(b + 1) * 128],
                             wv[:, k % 4, n * 512:(n + 1) * 512],
                             start=(k == 0), stop=(k == KT - 1))
        dst = vsb[b][:].rearrange("p (h c) -> p h c", c=65)[:, n * 8:(n + 1) * 8, 0:64]
        nc.vector.tensor_scalar_mul(dst, ps[:], rT[:, b:b + 1])

    # ---- Phase C: attention, software-pipelined with v/rope as filler ----
    oun = [apool.tile([128, L], FP16, name=f"oun_{l}_{t}", tag=f"oun{t}")
           for t in range(8)]
    of = [spool.tile([128, L], FP16, name=f"of_{l}_{t}", tag=f"of{t}", bufs=1)
          for t in range(8)]
    est_store = {}

    def emit_scores(ti):
        tiles = []
        for kt in range(4):
            q0 = 128 * kt
            sta = ps_g.tile([128, L], FP32, name=f"st_{l}_{ti}a_{kt}", tag="g")
            stb = ps_g.tile([128, L], FP32, name=f"st_{l}_{ti}b_{kt}", tag="g")
            nc.tensor.matmul(sta[:, q0:], qf[8 + ti][0:64, kt * 128:(kt + 1) * 128],
                             qf[ti][0:64, q0:], start=True, stop=True)
            nc.tensor.matmul(stb[:, q0:], qf[8 + ti][64:128, kt * 128:(kt + 1) * 128],
                             qf[ti][64:128, q0:], start=True, stop=True)
            esta = spool.tile([128, L], FP16, name=f"est_{l}_{ti}a_{kt}",
                              tag="esta", bufs=12)
            estb = spool.tile([128, L], FP16, name=f"est_{l}_{ti}b_{kt}",
                              tag="estb", bufs=12)
            nc.scalar.activation(esta[:, q0:], sta[:, q0:], F.Exp, scale=0.125)
            nc.scalar.activation(estb[:, q0:], stb[:, q0:], F.Exp, scale=0.125)
            nc.gpsimd.memset(esta[64:128, q0:q0 + 64], 0.0)
            nc.gpsimd.memset(estb[64:128, q0:q0 + 64], 0.0)
            tiles.append((esta, estb))
        est_store[ti] = tiles

    def emit_av(ti):
        ha, hb = 2 * ti, 2 * ti + 1
        oea = ps_acc.tile([65, L], FP32, name=f"oe_{l}_{ha}", tag="acc")
        oeb = ps_acc.tile([65, L], FP32, name=f"oe_{l}_{hb}", tag="acc")
        tiles = est_store.pop(ti)
        for kt in range(4):
            q0 = 128 * kt
            esta, estb = tiles[kt]
            nc.tensor.matmul(oea[:, q0:], vsb[kt][:, ha * 65:(ha + 1) * 65],
                             esta[:, q0:], start=(kt == 0), stop=(kt == 3))
            nc.tensor.matmul(oeb[:, q0:], vsb[kt][:, hb * 65:(hb + 1) * 65],
                             estb[:, q0:], start=(kt == 0), stop=(kt == 3))
        # denominator reciprocal via exp(-ln(den)) on [33, L] staging (both
        # functions live in the resident natlog_exp table set)
        den = spool.tile([33, L], FP32, name=f"den_{l}_{ti}", tag="den",
                         bufs=2)
        nc.gpsimd.memset(den[:], 1.0)
        nc.vector.tensor_copy(den[0:1, :], oea[64:65, :])
        nc.vector.tensor_copy(den[32:33, :], oeb[64:65, :])
        lnd = spool.tile([33, L], FP32, name=f"lnd_{l}_{ti}", tag="lnd",
                         bufs=2)
        nc.scalar.activation(lnd[:], den[:], F.Ln)
        rden = spool.tile([33, L], FP16, name=f"rden_{l}_{ti}", tag="rden",
                          bufs=2)
        nc.scalar.activation(rden[:], lnd[:], F.Exp, scale=-1.0)
        nc.vector.tensor_copy(oun[ti][0:64, :], oea[0:64, :])
        nc.vector.tensor_copy(oun[ti][64:128, :], oeb[0:64, :])
        # broadcast 1/den over partitions; 2-way packed (rows 0-63 / 64-127)
        bc = ps_bc.tile([128, L], FP32, name=f"nbc_{l}_{ti}", tag="bc")
        nc.tensor.matmul(bc[0:64, :], o1c[0:1, :], rden[0:1, :],
                         start=True, stop=True, tile_position=(0, 0))
        nc.tensor.matmul(bc[64:128, :], o1c[32:33, :], rden[32:33, :],
                         start=True, stop=True, tile_position=(32, 64))
        nc.vector.tensor_mul(of[ti][:], oun[ti][:], bc[:])

    # v + rope first (dense filler while alpha/stats chains settle), then
    # the score/attn-v pipeline with a 2-stage exp lead.
    rope_order = [x for p in zip(range(8), range(8, 16)) for x in p]
    vchunks = [(b, n) for n in range(2) for b in range(4)]
    for i in range(8):
        emit_rope(rope_order[2 * i])
        emit_rope(rope_order[2 * i + 1])
        emit_v(*vchunks[i])
    emit_scores(0)
    emit_scores(1)
    for ti in range(2, 8):
        emit_av(ti - 2)
        emit_scores(ti)
    emit_av(6)
    emit_av(7)

    # ---- Phase D: out projection + residual + mean2 ----
    mean2 = ps_acc.tile([1, L], FP32, name=f"mean2_{l}", tag="acc")
    for grp in range(2):
        wt = wpool.tile([128, 4, D], FP16, name=f"wo_{l}_{grp}", tag="w")
        nc.sync.dma_start(wt[:], d["wo_t"][l, grp * 4:(grp + 1) * 4]
                          .rearrange("g p n -> p g n"))
        for gi in range(4):
            t = grp * 4 + gi
            ps = ps_g.tile([128, L], FP32, name=f"xa_ps_{l}_{t}", tag="g")
            for k in range(KT):
                nc.tensor.matmul(ps[:], wt[:, gi, k * 128:(k + 1) * 128],
                                 of[k][:], start=(k == 0), stop=(k == KT - 1))
            nc.vector.scalar_tensor_tensor(x16[t][:], ps[:], 0.0, xs[t][:],
                                           ALU.add, ALU.add)
            nc.vector.tensor_add(xs[t][:], xs[t][:], ps[:])
            emit_mean(x16[t][:], t, mean2[:], f"m2_{l}")

    # ---- Phase E: MLP ----
    r2 = rms_recip(mean2[:], f"r2_{l}")
    h2 = [apool.tile([128, L], FP16, name=f"h2_{l}_{t}", tag=f"h2{t}")
          for t in range(8)]
    bch = ps_bc.tile([128, L], FP32, name=f"bch_{l}", tag="bc")
    nc.tensor.matmul(bch[:], ones1[:], r2[:], start=True, stop=True)
    bchh = spool.tile([128, L], FP16, name=f"bchh_{l}", tag="bchh", bufs=1)
    nc.vector.tensor_copy(bchh[:], bch[:])
    for t in range(8):
        nc.vector.tensor_mul(h2[t][:], x16[t][:], bchh[:])

    pj = []
    for j in range(JT):
        wt = wpool.tile([128, 4, D], FP16, name=f"gu_{l}_{j}", tag="w")
        nc.sync.dma_start(wt[:], d["gu_t"][l, j * 4:(j + 1) * 4]
                          .rearrange("g p n -> p g n"))
        # consume each psum right after its group (ps_g rotates 2 slots)
        sg1 = spool.tile([128, L], FP16, name=f"sg1_{l}_{j}", tag="sg1")
        sg2 = spool.tile([128, L], FP16, name=f"sg2_{l}_{j}", tag="sg2")
        ta = spool.tile([128, L], FP16, name=f"ta_{l}_{j}", tag="ta")
        tb = spool.tile([128, L], FP16, name=f"tb_{l}_{j}", tag="tb")
        for gi in range(4):
            ps = ps_g.tile([128, L], FP32, name=f"gu_ps_{l}_{j}_{gi}", tag="g")
            for k in range(KT):
                nc.tensor.matmul(ps[:], wt[:, gi, k * 128:(k + 1) * 128],
                                 h2[k][:], start=(k == 0), stop=(k == KT - 1))
            if gi == 0:
                nc.scalar.activation(sg1[:], ps[:], F.Sigmoid if SIM_SAFE else F.Silu)
            elif gi == 1:
                nc.scalar.activation(sg2[:], ps[:], F.Sigmoid if SIM_SAFE else F.Silu)
            elif gi == 2:
                nc.vector.scalar_tensor_tensor(ta[:], ps[:], 0.0, sg1[:],
                                               ALU.add, ALU.mult)
            else:
                nc.vector.scalar_tensor_tensor(tb[:], ps[:], 0.0, sg2[:],
                                               ALU.add, ALU.mult)
        p = spool.tile([128, L], FP16, name=f"p_{l}_{j}", tag=f"p{j}", bufs=1)
        nc.vector.tensor_add(p[:], ta[:], tb[:])
        pj.append(p)

    mean_next = ps_acc.tile([1, L], FP32, name=f"mean_{l + 1}", tag="acc")
    for grp in range(4):
        wt = wpool.tile([128, 2, IHP], FP16, name=f"dn_{l}_{grp}", tag="w")
        nc.sync.dma_start(wt[:], d["dn_t"][l, grp * 2:(grp + 1) * 2]
                          .rearrange("g p n -> p g n"))
        for gi in range(2):
            t = grp * 2 + gi
            ps = ps_g.tile([128, L], FP32, name=f"dn_ps_{l}_{t}", tag="g")
            for j in range(JT):
                nc.tensor.matmul(ps[:], wt[:, gi, j * 128:(j + 1) * 128],
                                 pj[j][:], start=(j == 0), stop=(j == JT - 1))
            nc.vector.scalar_tensor_tensor(x16[t][:], ps[:], 0.0, xs[t][:],
                                           ALU.add, ALU.add)
            nc.vector.tensor_add(xs[t][:], xs[t][:], ps[:])
            emit_mean(x16[t][:], t, mean_next[:], f"mn_{l}")
    return mean_next


# ----------------------------------------------------------------------
# entry point
# ----------------------------------------------------------------------

def _get_nc(nl=NL):
    if nl not in _CACHE:
        _CACHE[nl] = _build(nl)
    return _CACHE[nl]


def run(inputs, nl=NL, trace=False):
    inputs = {k: np.asarray(v) for k, v in inputs.items()}
    w = _prep_weights(inputs)
    in_maps = []
    for b in range(N_CORES):
        tok = _patchify(inputs["frames"][b]).astype(np.float32)
        x0T = np.zeros((PDP, L), np.float16)
        x0T[:PD] = tok.T.astype(np.float16)
        m = {"x0T": x0T, "qk_t": w["qk_t"], "wv_p": w["wv_p"],
             "wo_t": w["wo_t"], "gu_t": w["gu_t"], "dn_t": w["dn_t"],
             "pe_t": w["pe_t"], "hd_t": w["hd_t"],
             "e1": w["e1"], "esc": w["esc"],
             "cq": w["cq"], "sq": w["sq"], "ck": w["ck"], "sk": w["sk"],
             "b16": w["b16"], "psw": w["psw"]}
        in_maps.append(m)
    nc = _get_nc(nl)
    res = run_bass_kernel_spmd(nc, in_maps, list(range(N_CORES)), trace=trace)
    outs = []
    for b in range(N_CORES):
        tok = res.results[b]["out"].T  # (L, PD)
        outs.append(_unpatchify(tok))
    return np.stack(outs).astype(np.float32), res


def kernel(**inputs) -> np.ndarray:
    out, _ = run(inputs)
    return out



# revision 31
# speedup vs baseline: 1.0155x; 1.0007x over previous
"""AR video patch transformer forward on 8 Trainium2 NeuronCores.

Strategy: pure data parallelism — each core runs the full 8-layer
transformer on one batch element. Host does patchify/unpatchify and
weight preprocessing (scale folds, padding, lhsT tiling, fp16 cast).

v2 vs v1:
 - Emission reordered so the tensor queue stays dense (HAM clock-gate
   stays at 8/8 instead of oscillating to half clock in attention).
 - Attention softmax denominators: DVE reciprocal_approx_fast instead of
   a scalar Ln -> table swap -> Exp chain (removes a ~6.4us/layer stall).
 - RoPE pair-swap via DVE stream_shuffle (was a PE matmul).
 - All scalar rsqrt via Ln+Exp; a post-compile pass rewrites activation
   table-set ids to the combined natural_log_exp set and drops redundant
   loads (74 -> ~18 table loads).
 - Aux matmuls 2-way packed on the PE array (msq q/k, rope alpha bcast).
 - Fused psum-consume ops on DVE (scalar_tensor_tensor) for residual
   adds and gate*up products.

v3 vs v2:
 - MLP gate/up psums consumed immediately after each accumulation group
   (safe under a 2-slot psum rotation; keeps the PE queue dense).
 - r2/rf broadcast staged to fp16 once, so the 8 h2/hN row scalings run
   at 2x DVE rate instead of re-reading the fp32 psum broadcast.
 - Known-dead-end notes (all measured on HW, full 8-layer model;
   checkpoint reference ~1.87-1.89ms, run noise +-15us):
   * fp8 is out of error budget (absmax/rms gate 2e-2, fp16 sits at
     4e-3, one e4m3 GEMM costs ~3%).
   * GpSimd cannot read PSUM (BIR verifier); DVE reciprocal custom ops
     require SBUF fp32 in/out; moving the softmax-den reciprocal to DVE
     is therefore a wash (staging copies eat the Scalar savings).
   * Batching the den Ln/Exp across ti-pairs: 2063us (stalls attn-v).
   * Packed per-half est tiles ([128,1280], kt segments at col offsets)
     with per-kt psums/exps: runs correctly, 1899us (-0) — the
     coarser subtile deps eat the instruction savings.
   * + kt1/kt3 packed in one psum bank, 6 exps/ti: 1914us.
   * + kt0/kt1/kt3 in a [128,1024] 2-bank psum, 4 exps/ti (needs ps_g
     bufs=2): 1999us — the 2-slot rotation serializes half-b scores
     behind half-a's 1us exp, and PSUM (8 banks) can't fund deeper
     rotation.  2-bank psum tiles + bank-crossing ACT reads themselves
     are LEGAL and run fine.
   * The original runtime fault in earlier packed-scores attempts was
     the kt2 psum/est tile SHARED between halves combined with the
     (0,1,3,2) attn-v accumulate order — avoid sharing one psum tile
     between both head-halves' score groups.
   * scores->attn-v software-pipeline lead 3 (vs 2): 1905us.
   Remaining theoretical floor ~1.45-1.5ms: needs attention-phase PE
   density (HAM stays warm) without losing the baseline's fine-grained
   per-kt exp overlap — e.g. a token-halved layer pipeline that
   overlaps attention's Scalar/DVE chains with the MLP GEMM stream.
"""

import numpy as np

import concourse.bass as bass
import concourse.mybir as mybir
from concourse import bacc
from concourse.tile import TileContext
from concourse.bass_utils import run_bass_kernel_spmd

F = mybir.ActivationFunctionType
ALU = mybir.AluOpType
FP16 = mybir.dt.float16
FP32 = mybir.dt.float32

# Model config (hardcoded from the problem spec)
B = 8; T = 8; C = 3; RES = 64; P = 8
D = 1024; NH = 16; HD = 64; NL = 8
INNER = 2730
NP_ = 64           # patches per frame
PD = 192           # patch dim
PDP = 256          # padded patch dim (2 k-tiles)
L = 512            # tokens
EPS = 1e-6
KT = D // 128      # 8
IH = INNER // 2    # 1365 half-inner
IHP = 1408         # padded half-inner (11 tiles)
JT = IHP // 128    # 11
EP9 = 2.0 ** -9    # exact fp16 scalar used for the eps matmul

N_CORES = 8
_CACHE = {}

import os
DBG_NO_TBLFIX = os.environ.get("K_NO_TBLFIX", "") == "1"
DBG_NO_SHUF = os.environ.get("K_NO_SHUF", "") == "1"
DBG_NO_RECIP = os.environ.get("K_NO_RECIP", "") == "1"
# CoreSim has no Silu; K_SIMSAFE swaps in Sigmoid (structure-identical)
SIM_SAFE = os.environ.get("K_SIMSAFE", "") == "1"

SWAP_MASK = [i ^ 1 for i in range(32)]

# activation table sets (trn2/cayman act_info.json order)
SET_NATLOG_EXP = 6     # ln, exp, square, copy, ...
_REWRITE_SETS = {0, 5}  # exp_and_others / natural_log -> natlog_exp


class _Bacc(bacc.Bacc):
    """Bacc with a post-pass that merges ln/exp table sets and deletes
    redundant table loads (the stock pass picks the first set containing
    each function, so ln<->exp sequences thrash)."""

    def insert_act_table_loads(self):
        super().insert_act_table_loads()
        if DBG_NO_TBLFIX:
            return
        for blk in self.main_func.blocks:
            resident = None
            keep = []
            for inst in blk.instructions:
                if isinstance(inst, mybir.InstLoadActFuncSet):
                    if inst.act_func_set_id in _REWRITE_SETS:
                        inst.act_func_set_id = SET_NATLOG_EXP
                    if inst.act_func_set_id == resident and not inst.sync_info:
                        continue  # redundant, drop
                    resident = inst.act_func_set_id
                keep.append(inst)
            blk.instructions[:] = keep


# ----------------------------------------------------------------------
# host-side preprocessing
# ----------------------------------------------------------------------

def _lhsT_tile(w):
    """[Din, Dout] -> [Dout/128, 128, Din] fp16 lhsT-tiled blocks."""
    din, dout = w.shape
    kt, ot = din // 128, dout // 128
    return np.ascontiguousarray(
        w.reshape(kt, 128, ot, 128).transpose(2, 1, 0, 3).reshape(ot, 128, din)
    ).astype(np.float16)


def _rope_tables(scale):
    """C/S tables [128, L] with the per-dim norm scale folded in
    (scale applied before rotation, matching the reference order)."""
    q = HD // 4  # 16
    inv = 1.0 / (10000.0 ** (np.arange(q, dtype=np.float64) / q))
    t_idx = np.repeat(np.arange(T), NP_)
    s_idx = np.tile(np.arange(NP_), T)
    ang = np.concatenate(
        [t_idx[:, None] * inv[None, :], s_idx[:, None] * inv[None, :]], axis=1
    )  # (L, 32)
    cdm = np.zeros((128, L), np.float64)
    sdm = np.zeros((128, L), np.float64)
    for d in range(128):
        dl = d % 64
        i = dl // 2
        cdm[d] = np.cos(ang[:, i]) * scale[dl]
        sg = -1.0 if d % 2 == 0 else 1.0
        sdm[d] = sg * np.sin(ang[:, i]) * scale[dl ^ 1]
    return cdm.astype(np.float16), sdm.astype(np.float16)


def _prep_weights(inp):
    w = {}
    n1 = inp["norm1_scale"]; n2 = inp["norm2_scale"]
    qk_t = np.empty((NL, 16, 128, D), np.float16)
    wv_p = np.empty((NL, KT, 128, D), np.float16)
    wo_t = np.empty((NL, 8, 128, D), np.float16)
    gu_t = np.empty((NL, 44, 128, D), np.float16)
    dn_t = np.empty((NL, 8, 128, IHP), np.float16)
    for l in range(NL):
        w1 = inp["qkv_w"][l] * n1[l][:, None]
        qk_t[l] = _lhsT_tile(w1[:, :2048])
        wv_p[l] = w1[:, 2048:].reshape(KT, 128, D).astype(np.float16)
        wo_t[l] = _lhsT_tile(inp["out_w"][l])
        g = inp["gate_w"][l] * n2[l][:, None]
        u = inp["up_w"][l] * n2[l][:, None]
        gp = np.zeros((D, 2 * IHP), np.float32)
        up = np.zeros((D, 2 * IHP), np.float32)
        gp[:, :IH] = g[:, :IH]; gp[:, IHP:IHP + IH] = g[:, IH:]
        up[:, :IH] = u[:, :IH]; up[:, IHP:IHP + IH] = u[:, IH:]
        gt = _lhsT_tile(gp); ut = _lhsT_tile(up)
        order = []
        for j in range(JT):
            order += [gt[j], gt[JT + j], ut[j], ut[JT + j]]
        gu_t[l] = np.stack(order)
        dp = np.zeros((IHP, D), np.float32)
        dp[:IH] = inp["down_w"][l]
        dn_t[l] = _lhsT_tile(dp)
    w["qk_t"] = qk_t; w["wv_p"] = wv_p; w["wo_t"] = wo_t
    w["gu_t"] = gu_t; w["dn_t"] = dn_t

    pe = np.zeros((PDP, D), np.float32)
    pe[:PD] = inp["patch_embed_w"]
    w["pe_t"] = _lhsT_tile(pe)
    hw = np.zeros((D, PDP), np.float32)
    hw[:, :PD] = inp["head_w"] * inp["normf_scale"][:, None]
    w["hd_t"] = _lhsT_tile(hw)

    # per-head broadcast matrix (block ones); q rows 0-15, k rows 32-47
    e1 = np.zeros((48, 8, 128), np.float16)
    for t in range(8):
        for dl in range(128):
            e1[2 * t + dl // 64, t, dl] = 1.0
            e1[32 + 2 * t + dl // 64, t, dl] = 1.0
    w["e1"] = np.ascontiguousarray(e1.reshape(48, 8 * 128))

    w["esc"] = inp["embed_norm_scale"].reshape(1, 8 * 128).astype(np.float16)

    cq = np.empty((NL, 128, L), np.float16); sq = np.empty((NL, 128, L), np.float16)
    ck = np.empty((NL, 128, L), np.float16); sk = np.empty((NL, 128, L), np.float16)
    for l in range(NL):
        cq[l], sq[l] = _rope_tables(inp["q_norm_scale"][l])
        ck[l], sk[l] = _rope_tables(inp["k_norm_scale"][l])
    w["cq"] = cq; w["sq"] = sq; w["ck"] = ck; w["sk"] = sk
    b16 = np.zeros((128, 8 * 16), np.float16)
    for t in range(8):
        for dl in range(128):
            b16[dl, t * 16 + 2 * t + dl // 64] = 1.0 / 64.0
    w["b16"] = b16
    psw = np.zeros((128, 128), np.float16)
    for i in range(128):
        psw[i ^ 1, i] = 1.0
    w["psw"] = psw
    return w


def _patchify(frames_b):
    # (T, C, RES, RES) -> (L, PD)
    h = RES // P
    x = frames_b.reshape(T, C, h, P, h, P)
    x = x.transpose(0, 2, 4, 1, 3, 5).reshape(T * h * h, C * P * P)
    return x


def _unpatchify(tokens):
    # (L, PD) -> (T, C, RES, RES)
    h = RES // P
    y = tokens.reshape(T, h, h, C, P, P)
    return y.transpose(0, 3, 1, 4, 2, 5).reshape(T, C, RES, RES)


# ----------------------------------------------------------------------
# device kernel
# ----------------------------------------------------------------------

def _build(nl=NL):
    nc = _Bacc()
    d = {}
    d["x0T"] = nc.dram_tensor("x0T", [PDP, L], FP16, kind="ExternalInput")
    d["qk_t"] = nc.dram_tensor("qk_t", [NL, 16, 128, D], FP16, kind="ExternalInput")
    d["wv_p"] = nc.dram_tensor("wv_p", [NL, KT, 128, D], FP16, kind="ExternalInput")
    d["wo_t"] = nc.dram_tensor("wo_t", [NL, 8, 128, D], FP16, kind="ExternalInput")
    d["gu_t"] = nc.dram_tensor("gu_t", [NL, 44, 128, D], FP16, kind="ExternalInput")
    d["dn_t"] = nc.dram_tensor("dn_t", [NL, 8, 128, IHP], FP16, kind="ExternalInput")
    d["pe_t"] = nc.dram_tensor("pe_t", [8, 128, PDP], FP16, kind="ExternalInput")
    d["hd_t"] = nc.dram_tensor("hd_t", [2, 128, D], FP16, kind="ExternalInput")
    d["e1"] = nc.dram_tensor("e1", [48, 8 * 128], FP16, kind="ExternalInput")
    d["esc"] = nc.dram_tensor("esc", [1, 8 * 128], FP16, kind="ExternalInput")
    d["cq"] = nc.dram_tensor("cq", [NL, 128, L], FP16, kind="ExternalInput")
    d["sq"] = nc.dram_tensor("sq", [NL, 128, L], FP16, kind="ExternalInput")
    d["ck"] = nc.dram_tensor("ck", [NL, 128, L], FP16, kind="ExternalInput")
    d["sk"] = nc.dram_tensor("sk", [NL, 128, L], FP16, kind="ExternalInput")
    d["b16"] = nc.dram_tensor("b16", [128, 8 * 16], FP16, kind="ExternalInput")
    d["psw"] = nc.dram_tensor("psw", [128, 128], FP16, kind="ExternalInput")
    out_d = nc.dram_tensor("out", [PD, L], FP32, kind="ExternalOutput")

    with TileContext(nc) as tc:
        _emit(nc, tc, d, out_d, nl)
    nc.compile()
    return nc


def _emit(nc, tc, d, out_d, nl):
    import contextlib
    ctx = contextlib.ExitStack()
    with ctx:
        cpool = ctx.enter_context(tc.tile_pool(name="consts", bufs=1))
        xpool = ctx.enter_context(tc.tile_pool(name="x", bufs=1))
        wpool = ctx.enter_context(tc.tile_pool(name="w", bufs=3))
        apool = ctx.enter_context(tc.tile_pool(name="act", bufs=1))
        spool = ctx.enter_context(tc.tile_pool(name="small", bufs=2))
        # psum: tag "g" slots are 2 banks wide ([128,1024] fp32) so the
        # attention scores for one head-pair half (kt0+kt1+kt3 packed) fit
        # one slot; general [128,512] psums use half a slot.  2(g)*2 + 2(acc)
        # + 2(bc) = 8 banks exactly.
        ps_g = ctx.enter_context(tc.tile_pool(name="psg", bufs=4, space="PSUM"))
        ps_acc = ctx.enter_context(tc.tile_pool(name="psacc", bufs=2, space="PSUM"))
        ps_bc = ctx.enter_context(tc.tile_pool(name="psbc", bufs=2, space="PSUM"))

        # ---- persistent constants ----
        b16 = cpool.tile([128, 128], FP16, name="b16")
        nc.sync.dma_start(b16[:], d["b16"][:])
        pswc = cpool.tile([128, 128], FP16, name="pswc")
        nc.sync.dma_start(pswc[:], d["psw"][:])
        e1s = cpool.tile([48, 8 * 128], FP16, name="e1s")
        nc.sync.dma_start(e1s[:], d["e1"][:])
        escs = cpool.tile([1, 8 * 128], FP16, name="escs")
        nc.sync.dma_start(escs[:], d["esc"][:])
        ones1 = cpool.tile([1, 128], FP16, name="ones1")
        nc.gpsimd.memset(ones1[:], 1.0)
        o1c = cpool.tile([33, 64], FP16, name="o1c")
        nc.gpsimd.memset(o1c[:], 1.0)
        onesd = cpool.tile([128, 1], FP16, name="onesd")
        nc.gpsimd.memset(onesd[:], 1.0 / 1024.0)
        ep9 = cpool.tile([1, 16], FP16, name="ep9")
        nc.gpsimd.memset(ep9[:], EP9)
        epsb = cpool.tile([128, 1], FP32, name="epsb")
        nc.gpsimd.memset(epsb[:], EPS)
        identf = cpool.tile([1, 1], FP32, name="identf")
        nc.gpsimd.memset(identf[:], 1.0)

        # residual stream
        xs = [xpool.tile([128, L], FP32, name=f"x{t}") for t in range(8)]
        x16 = [xpool.tile([128, L], FP16, name=f"m{t}") for t in range(8)]

        def rms_recip(mean_ps, name, dtype=FP16):
            """[1, L] psum mean -> [1, L] rsqrt(mean+eps) via Ln+Exp."""
            lnm = spool.tile([1, L], FP32, name=f"lnm_{name}", tag="r32")
            nc.scalar.activation(lnm[:], mean_ps[:], F.Ln, bias=epsb[0:1])
            r = spool.tile([1, L], dtype, name=f"r_{name}", tag="r16")
            nc.scalar.activation(r[:], lnm[:], F.Exp, scale=-0.5)
            return r

        def emit_mean(src16, t, mean_ps, name):
            """accumulate mean(src16^2) over feature tiles into [1, L] psum."""
            sq = spool.tile([128, L], FP16, name=f"sq_{name}_{t}", tag="sq",
                            bufs=2)
            nc.vector.tensor_mul(sq[:], src16[:], src16[:])
            nc.tensor.matmul(mean_ps[:], onesd[:], sq[:],
                             start=(t == 0), stop=(t == 7))

        # ---------- patch embed ----------
        x0 = apool.tile([128, 2, L], FP16, name="x0")
        nc.sync.dma_start(x0[:], d["x0T"].rearrange("(k p) t -> p k t", p=128))
        mean_e = ps_acc.tile([1, L], FP32, name="mean_e", tag="acc")
        for t in range(8):
            wt = wpool.tile([128, PDP], FP16, name=f"pe_w{t}", tag="w")
            nc.sync.dma_start(wt[:], d["pe_t"][t])
            ps = ps_g.tile([128, L], FP32, name=f"pe_ps{t}", tag="g")
            for k in range(2):
                nc.tensor.matmul(ps[:], wt[:, k * 128:(k + 1) * 128],
                                 x0[:, k, :], start=(k == 0), stop=(k == 1))
            nc.scalar.activation(xs[t][:], ps[:], F.Copy)
            nc.vector.tensor_copy(x16[t][:], xs[t][:])
            sqe = spool.tile([128, L], FP16, name=f"sqe_{t}", tag="sq", bufs=2)
            nc.scalar.activation(sqe[:], ps[:], F.Square)
            nc.tensor.matmul(mean_e[:], onesd[:], sqe[:],
                             start=(t == 0), stop=(t == 7))
        re_sb = rms_recip(mean_e[:], "emb")
        mean_in = ps_acc.tile([1, L], FP32, name="mean_l0", tag="acc")
        for t in range(8):
            bc = ps_bc.tile([128, L], FP32, name=f"bc_emb_{t}", tag="bc")
            nc.tensor.matmul(bc[:], escs[0:1, t * 128:(t + 1) * 128], re_sb[:],
                             start=True, stop=True)
            nc.vector.tensor_mul(xs[t][:], xs[t][:], bc[:])
            nc.vector.tensor_copy(x16[t][:], xs[t][:])
            emit_mean(x16[t][:], t, mean_in[:], "l0")

        # ---------- layers ----------
        for l in range(nl):
            mean_in = _layer(nc, tc, d, l, xs, x16, cpool, wpool, apool, spool,
                             ps_g, ps_acc, ps_bc,
                             b16, e1s, ones1, o1c, onesd, ep9, epsb, identf,
                             rms_recip, mean_in, emit_mean, pswc)

        # ---------- final norm + head ----------
        rf_sb = rms_recip(mean_in[:], "fin")
        hN = [apool.tile([128, L], FP16, name=f"hN{t}", tag=f"h2{t}")
              for t in range(8)]
        bcf = ps_bc.tile([128, L], FP32, name="bc_fin", tag="bc")
        nc.tensor.matmul(bcf[:], ones1[:], rf_sb[:], start=True, stop=True)
        bcfh = spool.tile([128, L], FP16, name="bcfh", tag="bchh", bufs=1)
        nc.vector.tensor_copy(bcfh[:], bcf[:])
        for t in range(8):
            nc.vector.tensor_mul(hN[t][:], x16[t][:], bcfh[:])
        for o in range(2):
            wt = wpool.tile([128, D], FP16, name=f"hd_w{o}", tag="w")
            nc.sync.dma_start(wt[:], d["hd_t"][o])
            ps = ps_g.tile([128, L], FP32, name=f"hd_ps{o}", tag="g")
            for k in range(KT):
                nc.tensor.matmul(ps[:], wt[:, k * 128:(k + 1) * 128],
                                 hN[k][:], start=(k == 0), stop=(k == KT - 1))
            rows = 128 if o == 0 else PD - 128
            ot = apool.tile([128, L], FP32, name=f"hd_o{o}")
            nc.scalar.activation(ot[:rows, :], ps[:rows, :], F.Copy)
            nc.sync.dma_start(out_d[o * 128:o * 128 + rows, :], ot[:rows, :])


def _layer(nc, tc, d, l, xs, x16, cpool, wpool, apool, spool,
           ps_g, ps_acc, ps_bc,
           b16, e1s, ones1, o1c, onesd, ep9, epsb, identf, rms_recip,
           mean_in, emit_mean, pswc):
    # per-layer rope/scale tables (prefetched; DMA engine is idle)
    rtab = spool.tile([128, 4, L], FP16, name=f"rtab_{l}", tag="rtab", bufs=2)
    nc.sync.dma_start(rtab[:, 0, :], d["cq"][l])
    nc.sync.dma_start(rtab[:, 1, :], d["sq"][l])
    nc.sync.dma_start(rtab[:, 2, :], d["ck"][l])
    nc.sync.dma_start(rtab[:, 3, :], d["sk"][l])

    # mean1-derived values (mean_in accumulated at the previous layer's tail)
    zs = spool.tile([1, L], FP16, name=f"zs_{l}", tag="zs")
    nc.vector.tensor_scalar_mul(zs[:], mean_in[:], EPS / EP9)
    r1 = rms_recip(mean_in[:], f"r1_{l}", dtype=FP32)
    rT = spool.tile([128, 4], FP32, name=f"rT_{l}", tag="rT")

    # msq accumulators: q at psum partitions 0-15 (bank 1), k at partitions
    # 32-47 (bank 2) so the accumulation matmuls pair up on disjoint PE
    # column groups and run concurrently.
    msq_q = ps_acc.tile([16, L], FP32, name=f"msqq_{l}", tag="acc")
    msq_k = ps_acc.tile([48, L], FP32, name=f"msqk_{l}", tag="acc")

    # ---- Phase A: qkv q/k GEMM + statistics, tensor queue kept dense ----
    qraw = []
    for grp in range(4):
        if grp == 1:
            # transpose r1 -> rT [128, 4] (per-token scale for token-major
            # v); emitted behind the first qkv chains so the tensor queue
            # head never blocks on the scalar r1 chain.
            trp = ps_bc.tile([128, 4], FP32, name=f"trp_{l}", tag="bc")
            for b in range(4):
                nc.tensor.transpose(trp[:, b:b + 1],
                                    r1[:, b * 128:(b + 1) * 128], identf[:])
            nc.vector.tensor_copy(rT[:], trp[:])
        wt = wpool.tile([128, 4, D], FP16, name=f"qkw_{l}_{grp}", tag="w")
        nc.sync.dma_start(wt[:], d["qk_t"][l, grp * 4:(grp + 1) * 4]
                          .rearrange("g p n -> p g n"))
        for gi in range(4):
            ot = grp * 4 + gi
            ps = ps_g.tile([128, L], FP32, name=f"qk_ps_{l}_{ot}", tag="g")
            for k in range(KT):
                nc.tensor.matmul(ps[:], wt[:, gi, k * 128:(k + 1) * 128],
                                 x16[k][:], start=(k == 0), stop=(k == KT - 1))
            qr = spool.tile([128, L], FP16, name=f"qraw_{l}_{ot}", tag="qraw",
                            bufs=16)
            nc.vector.tensor_copy(qr[:], ps[:])
            qraw.append(qr)
            sq = spool.tile([128, L], FP16, name=f"qsq_{l}_{ot}", tag="sq",
                            bufs=2)
            nc.vector.tensor_mul(sq[:], qr[:], qr[:])
            tt = ot % 8
            if ot < 8:
                nc.tensor.matmul(msq_q[:], b16[:, tt * 16:(tt + 1) * 16],
                                 sq[:], start=(tt == 0), stop=False,
                                 tile_position=(0, 0))
            else:
                nc.tensor.matmul(msq_k[32:48, :], b16[:, tt * 16:(tt + 1) * 16],
                                 sq[:], start=(tt == 0), stop=False,
                                 tile_position=(0, 32))

    # eps correction: msq += ep9 * zs  (= eps * ir2); 2-way packed
    nc.tensor.matmul(msq_q[:], ep9[:], zs[:], start=False, stop=True,
                     tile_position=(0, 0))
    nc.tensor.matmul(msq_k[32:48, :], ep9[:], zs[:], start=False, stop=True,
                     tile_position=(0, 32))

    # per-head q/k norm multipliers: alf rows 0-15 = q, rows 32-47 = k
    alf = spool.tile([48, L], FP16, name=f"alf_{l}", tag="alf")
    for row, msq in ((0, msq_q[:]), (32, msq_k[32:48, :])):
        tl = spool.tile([16, L], FP32, name=f"aln_{l}_{row}", tag="a32")
        nc.scalar.activation(tl[:], msq, F.Ln)
        nc.scalar.activation(alf[row:row + 16, :], tl[:], F.Exp, scale=-0.5)

    # ---- Phase B: v GEMM + rope, interleaved ----
    vsb = [apool.tile([128, 16 * 65], FP16, name=f"vsb_{l}_{b}", tag=f"vsb{b}")
           for b in range(4)]
    for b in range(4):
        nc.gpsimd.memset(
            vsb[b][:].rearrange("p (h c) -> p h c", c=65)[:, :, 64:65], 1.0)
    wva = wpool.tile([128, 4, D], FP16, name=f"vwa_{l}", tag="w")
    nc.sync.dma_start(wva[:], d["wv_p"][l, 0:4].rearrange("k p n -> p k n"))
    wvb = wpool.tile([128, 4, D], FP16, name=f"vwb_{l}", tag="w")
    nc.sync.dma_start(wvb[:], d["wv_p"][l, 4:8].rearrange("k p n -> p k n"))

    qf = [None] * 16

    def emit_rope(ot):
        t = ot % 8
        arow = 0 if ot < 8 else 32
        ci, si = (0, 1) if ot < 8 else (2, 3)
        bc = ps_bc.tile([128, L], FP32, name=f"rbc_{l}_{ot}", tag="bc")
        nc.tensor.matmul(bc[:], e1s[arow:arow + 16, t * 128:(t + 1) * 128],
                         alf[arow:arow + 16, :], start=True, stop=True)
        sw = spool.tile([128, L], FP16, name=f"rsw_{l}_{ot}", tag="u2")
        if DBG_NO_SHUF:
            swp = ps_bc.tile([128, L], FP32, name=f"rswp_{l}_{ot}", tag="bc")
            nc.tensor.matmul(swp[:], pswc[:], qraw[ot][:], start=True,
                             stop=True)
            nc.vector.tensor_copy(sw[:], swp[:])
        else:
            nc.vector.stream_shuffle(sw[:], qraw[ot][:], SWAP_MASK)
        u1 = spool.tile([128, L], FP16, name=f"u1_{l}_{ot}", tag="u1")
        nc.vector.tensor_mul(u1[:], qraw[ot][:], rtab[:, ci, :])
        u2 = spool.tile([128, L], FP16, name=f"u2_{l}_{ot}", tag="u1")
        nc.vector.tensor_mul(u2[:], sw[:], rtab[:, si, :])
        nc.vector.tensor_add(u1[:], u1[:], u2[:])
        qt = apool.tile([128, L], FP16, name=f"qf_{l}_{ot}", tag=f"qf{ot}")
        nc.vector.tensor_mul(qt[:], u1[:], bc[:])
        qf[ot] = qt

    def emit_v(b, n):
        ps = ps_g.tile([128, 512], FP32, name=f"v_ps_{l}_{b}_{n}", tag="g")
        for k in range(KT):
            wv = wva if k < 4 else wvb
            nc.tensor.matmul(ps[:], x16[k][:, b * 128:(b + 1) * 128],
                             wv[:, k % 4, n * 512:(n + 1) * 512],
                             start=(k == 0), stop=(k == KT - 1))
        dst = vsb[b][:].rearrange("p (h c) -> p h c", c=65)[:, n * 8:(n + 1) * 8, 0:64]
        nc.vector.tensor_scalar_mul(dst, ps[:], rT[:, b:b + 1])

    # ---- Phase C: attention, software-pipelined with v/rope as filler ----
    oun = [apool.tile([128, L], FP16, name=f"oun_{l}_{t}", tag=f"oun{t}")
           for t in range(8)]
    of = [spool.tile([128, L], FP16, name=f"of_{l}_{t}", tag=f"of{t}", bufs=1)
          for t in range(8)]
    est_store = {}

    def emit_scores(ti):
        tiles = []
        for kt in range(4):
            q0 = 128 * kt
            sta = ps_g.tile([128, L], FP32, name=f"st_{l}_{ti}a_{kt}", tag="g")
            stb = ps_g.tile([128, L], FP32, name=f"st_{l}_{ti}b_{kt}", tag="g")
            nc.tensor.matmul(sta[:, q0:], qf[8 + ti][0:64, kt * 128:(kt + 1) * 128],
                             qf[ti][0:64, q0:], start=True, stop=True)
            nc.tensor.matmul(stb[:, q0:], qf[8 + ti][64:128, kt * 128:(kt + 1) * 128],
                             qf[ti][64:128, q0:], start=True, stop=True)
            esta = spool.tile([128, L], FP16, name=f"est_{l}_{ti}a_{kt}",
                              tag="esta", bufs=12)
            estb = spool.tile([128, L], FP16, name=f"est_{l}_{ti}b_{kt}",
                              tag="estb", bufs=12)
            nc.scalar.activation(esta[:, q0:], sta[:, q0:], F.Exp, scale=0.125)
            nc.scalar.activation(estb[:, q0:], stb[:, q0:], F.Exp, scale=0.125)
            nc.gpsimd.memset(esta[64:128, q0:q0 + 64], 0.0)
            nc.gpsimd.memset(estb[64:128, q0:q0 + 64], 0.0)
            tiles.append((esta, estb))
        est_store[ti] = tiles

    def emit_av(ti):
        ha, hb = 2 * ti, 2 * ti + 1
        oea = ps_acc.tile([65, L], FP32, name=f"oe_{l}_{ha}", tag="acc")
        oeb = ps_acc.tile([65, L], FP32, name=f"oe_{l}_{hb}", tag="acc")
        tiles = est_store.pop(ti)
        for kt in range(4):
            q0 = 128 * kt
            esta, estb = tiles[kt]
            nc.tensor.matmul(oea[:, q0:], vsb[kt][:, ha * 65:(ha + 1) * 65],
                             esta[:, q0:], start=(kt == 0), stop=(kt == 3))
            nc.tensor.matmul(oeb[:, q0:], vsb[kt][:, hb * 65:(hb + 1) * 65],
                             estb[:, q0:], start=(kt == 0), stop=(kt == 3))
        # denominator reciprocal via exp(-ln(den)) on [33, L] staging (both
        # functions live in the resident natlog_exp table set)
        den = spool.tile([33, L], FP32, name=f"den_{l}_{ti}", tag="den",
                         bufs=2)
        nc.gpsimd.memset(den[:], 1.0)
        nc.vector.tensor_copy(den[0:1, :], oea[64:65, :])
        nc.vector.tensor_copy(den[32:33, :], oeb[64:65, :])
        lnd = spool.tile([33, L], FP32, name=f"lnd_{l}_{ti}", tag="lnd",
                         bufs=2)
        nc.scalar.activation(lnd[:], den[:], F.Ln)
        rden = spool.tile([33, L], FP16, name=f"rden_{l}_{ti}", tag="rden",
                          bufs=2)
        nc.scalar.activation(rden[:], lnd[:], F.Exp, scale=-1.0)
        nc.vector.tensor_copy(oun[ti][0:64, :], oea[0:64, :])
        nc.vector.tensor_copy(oun[ti][64:128, :], oeb[0:64, :])
        # broadcast 1/den over partitions; 2-way packed (rows 0-63 / 64-127)
        bc = ps_bc.tile([128, L], FP32, name=f"nbc_{l}_{ti}", tag="bc")
        nc.tensor.matmul(bc[0:64, :], o1c[0:1, :], rden[0:1, :],
                         start=True, stop=True, tile_position=(0, 0))
        nc.tensor.matmul(bc[64:128, :], o1c[32:33, :], rden[32:33, :],
                         start=True, stop=True, tile_position=(32, 64))
        nc.vector.tensor_mul(of[ti][:], oun[ti][:], bc[:])

    # v + rope first (dense filler while alpha/stats chains settle), then
    # the score/attn-v pipeline with a 2-stage exp lead.
    rope_order = [x for p in zip(range(8), range(8, 16)) for x in p]
    vchunks = [(b, n) for n in range(2) for b in range(4)]
    for i in range(8):
        emit_rope(rope_order[2 * i])
        emit_rope(rope_order[2 * i + 1])
        emit_v(*vchunks[i])
    emit_scores(0)
    emit_scores(1)
    for ti in range(2, 8):
        emit_av(ti - 2)
        emit_scores(ti)
    emit_av(6)
    emit_av(7)

    # ---- Phase D: out projection + residual + mean2 ----
    mean2 = ps_acc.tile([1, L], FP32, name=f"mean2_{l}", tag="acc")
    for grp in range(2):
        wt = wpool.tile([128, 4, D], FP16, name=f"wo_{l}_{grp}", tag="w")
        nc.sync.dma_start(wt[:], d["wo_t"][l, grp * 4:(grp + 1) * 4]
                          .rearrange("g p n -> p g n"))
        for gi in range(4):
            t = grp * 4 + gi
            ps = ps_g.tile([128, L], FP32, name=f"xa_ps_{l}_{t}", tag="g")
            for k in range(KT):
                nc.tensor.matmul(ps[:], wt[:, gi, k * 128:(k + 1) * 128],
                                 of[k][:], start=(k == 0), stop=(k == KT - 1))
            nc.vector.scalar_tensor_tensor(x16[t][:], ps[:], 0.0, xs[t][:],
                                           ALU.add, ALU.add)
            nc.vector.tensor_add(xs[t][:], xs[t][:], ps[:])
            emit_mean(x16[t][:], t, mean2[:], f"m2_{l}")

    # ---- Phase E: MLP ----
    r2 = rms_recip(mean2[:], f"r2_{l}")
    h2 = [apool.tile([128, L], FP16, name=f"h2_{l}_{t}", tag=f"h2{t}")
          for t in range(8)]
    bch = ps_bc.tile([128, L], FP32, name=f"bch_{l}", tag="bc")
    nc.tensor.matmul(bch[:], ones1[:], r2[:], start=True, stop=True)
    bchh = spool.tile([128, L], FP16, name=f"bchh_{l}", tag="bchh", bufs=1)
    nc.vector.tensor_copy(bchh[:], bch[:])
    for t in range(8):
        nc.vector.tensor_mul(h2[t][:], x16[t][:], bchh[:])

    pj = []
    for j in range(JT):
        wt = wpool.tile([128, 4, D], FP16, name=f"gu_{l}_{j}", tag="w")
        nc.sync.dma_start(wt[:], d["gu_t"][l, j * 4:(j + 1) * 4]
                          .rearrange("g p n -> p g n"))
        # consume each psum right after its group (ps_g rotates 2 slots)
        sg1 = spool.tile([128, L], FP16, name=f"sg1_{l}_{j}", tag="sg1")
        sg2 = spool.tile([128, L], FP16, name=f"sg2_{l}_{j}", tag="sg2")
        ta = spool.tile([128, L], FP16, name=f"ta_{l}_{j}", tag="ta")
        tb = spool.tile([128, L], FP16, name=f"tb_{l}_{j}", tag="tb")
        for gi in range(4):
            ps = ps_g.tile([128, L], FP32, name=f"gu_ps_{l}_{j}_{gi}", tag="g")
            for k in range(KT):
                nc.tensor.matmul(ps[:], wt[:, gi, k * 128:(k + 1) * 128],
                                 h2[k][:], start=(k == 0), stop=(k == KT - 1))
            if gi == 0:
                nc.scalar.activation(sg1[:], ps[:], F.Sigmoid if SIM_SAFE else F.Silu)
            elif gi == 1:
                nc.scalar.activation(sg2[:], ps[:], F.Sigmoid if SIM_SAFE else F.Silu)
            elif gi == 2:
                nc.vector.scalar_tensor_tensor(ta[:], ps[:], 0.0, sg1[:],
                                               ALU.add, ALU.mult)
            else:
                nc.vector.scalar_tensor_tensor(tb[:], ps[:], 0.0, sg2[:],
                                               ALU.add, ALU.mult)
        p = spool.tile([128, L], FP16, name=f"p_{l}_{j}", tag=f"p{j}", bufs=1)
        nc.vector.tensor_add(p[:], ta[:], tb[:])
        pj.append(p)

    mean_next = ps_acc.tile([1, L], FP32, name=f"mean_{l + 1}", tag="acc")
    for grp in range(4):
        wt = wpool.tile([128, 2, IHP], FP16, name=f"dn_{l}_{grp}", tag="w")
        nc.sync.dma_start(wt[:], d["dn_t"][l, grp * 2:(grp + 1) * 2]
                          .rearrange("g p n -> p g n"))
        for gi in range(2):
            t = grp * 2 + gi
            ps = ps_g.tile([128, L], FP32, name=f"dn_ps_{l}_{t}", tag="g")
            for j in range(JT):
                nc.tensor.matmul(ps[:], wt[:, gi, j * 128:(j + 1) * 128],
                                 pj[j][:], start=(j == 0), stop=(j == JT - 1))
            nc.vector.scalar_tensor_tensor(x16[t][:], ps[:], 0.0, xs[t][:],
                                           ALU.add, ALU.add)
            nc.vector.tensor_add(xs[t][:], xs[t][:], ps[:])
            emit_mean(x16[t][:], t, mean_next[:], f"mn_{l}")
    return mean_next


# ----------------------------------------------------------------------
# entry point
# ----------------------------------------------------------------------

def _get_nc(nl=NL):
    if nl not in _CACHE:
        _CACHE[nl] = _build(nl)
    return _CACHE[nl]


def run(inputs, nl=NL, trace=False):
    inputs = {k: np.asarray(v) for k, v in inputs.items()}
    w = _prep_weights(inputs)
    in_maps = []
    for b in range(N_CORES):
        tok = _patchify(inputs["frames"][b]).astype(np.float32)
        x0T = np.zeros((PDP, L), np.float16)
        x0T[:PD] = tok.T.astype(np.float16)
        m = {"x0T": x0T, "qk_t": w["qk_t"], "wv_p": w["wv_p"],
             "wo_t": w["wo_t"], "gu_t": w["gu_t"], "dn_t": w["dn_t"],
             "pe_t": w["pe_t"], "hd_t": w["hd_t"],
             "e1": w["e1"], "esc": w["esc"],
             "cq": w["cq"], "sq": w["sq"], "ck": w["ck"], "sk": w["sk"],
             "b16": w["b16"], "psw": w["psw"]}
        in_maps.append(m)
    nc = _get_nc(nl)
    res = run_bass_kernel_spmd(nc, in_maps, list(range(N_CORES)), trace=trace)
    outs = []
    for b in range(N_CORES):
        tok = res.results[b]["out"].T  # (L, PD)
        outs.append(_unpatchify(tok))
    return np.stack(outs).astype(np.float32), res


def kernel(**inputs) -> np.ndarray:
    out, _ = run(inputs)
    return out



# revision 39
# speedup vs baseline: 1.0191x; 1.0035x over previous
"""AR video patch transformer forward on 8 Trainium2 NeuronCores.

Strategy: pure data parallelism — each core runs the full 8-layer
transformer on one batch element. Host does patchify/unpatchify and
weight preprocessing (scale folds, padding, lhsT tiling, fp16 cast).

v2 vs v1:
 - Emission reordered so the tensor queue stays dense (HAM clock-gate
   stays at 8/8 instead of oscillating to half clock in attention).
 - Attention softmax denominators: DVE reciprocal_approx_fast instead of
   a scalar Ln -> table swap -> Exp chain (removes a ~6.4us/layer stall).
 - RoPE pair-swap via DVE stream_shuffle (was a PE matmul).
 - All scalar rsqrt via Ln+Exp; a post-compile pass rewrites activation
   table-set ids to the combined natural_log_exp set and drops redundant
   loads (74 -> ~18 table loads).
 - Aux matmuls 2-way packed on the PE array (msq q/k, rope alpha bcast).
 - Fused psum-consume ops on DVE (scalar_tensor_tensor) for residual
   adds and gate*up products.

v3 vs v2:
 - MLP gate/up psums consumed immediately after each accumulation group
   (safe under a 2-slot psum rotation; keeps the PE queue dense).
 - r2/rf broadcast staged to fp16 once, so the 8 h2/hN row scalings run
   at 2x DVE rate instead of re-reading the fp32 psum broadcast.
 - sq staging stays 3-deep (2-deep ping-pongs DVE<->PE on the mean-emit
   chain); lnd is 1-deep (Ln and Exp share the in-order Scalar queue).
   NOTE: run-to-run noise is up to ~55us (HAM phase / power state) —
   single-run comparisons under ~30us are meaningless; sample 3+ runs.
 - Known-dead-end notes: fusing the per-kt softmax exps via packed score
   psums ([128,1024] 2-bank or single-bank layouts) compiles + passes
   CoreSim but faults at runtime on HW; GpSimd cannot read PSUM (BIR
   verifier); DVE reciprocal custom ops require SBUF fp32 in/out;
   batching the den Ln/Exp across ti-pairs regresses ~190us (stalls the
   attn-v pipeline).  fp8 is out of error budget (absmax/rms gate 2e-2,
   fp16 sits at 4e-3, one e4m3 GEMM costs ~3%).
"""

import numpy as np

import concourse.bass as bass
import concourse.mybir as mybir
from concourse import bacc
from concourse.tile import TileContext
from concourse.bass_utils import run_bass_kernel_spmd

F = mybir.ActivationFunctionType
ALU = mybir.AluOpType
FP16 = mybir.dt.float16
FP32 = mybir.dt.float32

# Model config (hardcoded from the problem spec)
B = 8; T = 8; C = 3; RES = 64; P = 8
D = 1024; NH = 16; HD = 64; NL = 8
INNER = 2730
NP_ = 64           # patches per frame
PD = 192           # patch dim
PDP = 256          # padded patch dim (2 k-tiles)
L = 512            # tokens
EPS = 1e-6
KT = D // 128      # 8
IH = INNER // 2    # 1365 half-inner
IHP = 1408         # padded half-inner (11 tiles)
JT = IHP // 128    # 11
EP9 = 2.0 ** -9    # exact fp16 scalar used for the eps matmul

N_CORES = 8
_CACHE = {}

import os
DBG_NO_TBLFIX = os.environ.get("K_NO_TBLFIX", "") == "1"
DBG_NO_SHUF = os.environ.get("K_NO_SHUF", "") == "1"
DBG_NO_RECIP = os.environ.get("K_NO_RECIP", "") == "1"
# CoreSim has no Silu; K_SIMSAFE swaps in Sigmoid (structure-identical)
SIM_SAFE = os.environ.get("K_SIMSAFE", "") == "1"

SWAP_MASK = [i ^ 1 for i in range(32)]

# activation table sets (trn2/cayman act_info.json order)
SET_NATLOG_EXP = 6     # ln, exp, square, copy, ...
_REWRITE_SETS = {0, 5}  # exp_and_others / natural_log -> natlog_exp


class _Bacc(bacc.Bacc):
    """Bacc with a post-pass that merges ln/exp table sets and deletes
    redundant table loads (the stock pass picks the first set containing
    each function, so ln<->exp sequences thrash)."""

    def insert_act_table_loads(self):
        super().insert_act_table_loads()
        if DBG_NO_TBLFIX:
            return
        for blk in self.main_func.blocks:
            resident = None
            keep = []
            for inst in blk.instructions:
                if isinstance(inst, mybir.InstLoadActFuncSet):
                    if inst.act_func_set_id in _REWRITE_SETS:
                        inst.act_func_set_id = SET_NATLOG_EXP
                    if inst.act_func_set_id == resident and not inst.sync_info:
                        continue  # redundant, drop
                    resident = inst.act_func_set_id
                keep.append(inst)
            blk.instructions[:] = keep


# ----------------------------------------------------------------------
# host-side preprocessing
# ----------------------------------------------------------------------

def _lhsT_tile(w):
    """[Din, Dout] -> [Dout/128, 128, Din] fp16 lhsT-tiled blocks."""
    din, dout = w.shape
    kt, ot = din // 128, dout // 128
    return np.ascontiguousarray(
        w.reshape(kt, 128, ot, 128).transpose(2, 1, 0, 3).reshape(ot, 128, din)
    ).astype(np.float16)


def _rope_tables(scale):
    """C/S tables [128, L] with the per-dim norm scale folded in
    (scale applied before rotation, matching the reference order)."""
    q = HD // 4  # 16
    inv = 1.0 / (10000.0 ** (np.arange(q, dtype=np.float64) / q))
    t_idx = np.repeat(np.arange(T), NP_)
    s_idx = np.tile(np.arange(NP_), T)
    ang = np.concatenate(
        [t_idx[:, None] * inv[None, :], s_idx[:, None] * inv[None, :]], axis=1
    )  # (L, 32)
    cdm = np.zeros((128, L), np.float64)
    sdm = np.zeros((128, L), np.float64)
    for d in range(128):
        dl = d % 64
        i = dl // 2
        cdm[d] = np.cos(ang[:, i]) * scale[dl]
        sg = -1.0 if d % 2 == 0 else 1.0
        sdm[d] = sg * np.sin(ang[:, i]) * scale[dl ^ 1]
    return cdm.astype(np.float16), sdm.astype(np.float16)


def _prep_weights(inp):
    w = {}
    n1 = inp["norm1_scale"]; n2 = inp["norm2_scale"]
    qk_t = np.empty((NL, 16, 128, D), np.float16)
    wv_p = np.empty((NL, KT, 128, D), np.float16)
    wo_t = np.empty((NL, 8, 128, D), np.float16)
    gu_t = np.empty((NL, 44, 128, D), np.float16)
    dn_t = np.empty((NL, 8, 128, IHP), np.float16)
    for l in range(NL):
        w1 = inp["qkv_w"][l] * n1[l][:, None]
        qk_t[l] = _lhsT_tile(w1[:, :2048])
        wv_p[l] = w1[:, 2048:].reshape(KT, 128, D).astype(np.float16)
        wo_t[l] = _lhsT_tile(inp["out_w"][l])
        g = inp["gate_w"][l] * n2[l][:, None]
        u = inp["up_w"][l] * n2[l][:, None]
        gp = np.zeros((D, 2 * IHP), np.float32)
        up = np.zeros((D, 2 * IHP), np.float32)
        gp[:, :IH] = g[:, :IH]; gp[:, IHP:IHP + IH] = g[:, IH:]
        up[:, :IH] = u[:, :IH]; up[:, IHP:IHP + IH] = u[:, IH:]
        gt = _lhsT_tile(gp); ut = _lhsT_tile(up)
        order = []
        for j in range(JT):
            order += [gt[j], gt[JT + j], ut[j], ut[JT + j]]
        gu_t[l] = np.stack(order)
        dp = np.zeros((IHP, D), np.float32)
        dp[:IH] = inp["down_w"][l]
        dn_t[l] = _lhsT_tile(dp)
    w["qk_t"] = qk_t; w["wv_p"] = wv_p; w["wo_t"] = wo_t
    w["gu_t"] = gu_t; w["dn_t"] = dn_t

    pe = np.zeros((PDP, D), np.float32)
    pe[:PD] = inp["patch_embed_w"]
    w["pe_t"] = _lhsT_tile(pe)
    hw = np.zeros((D, PDP), np.float32)
    hw[:, :PD] = inp["head_w"] * inp["normf_scale"][:, None]
    w["hd_t"] = _lhsT_tile(hw)

    # per-head broadcast matrix (block ones); q rows 0-15, k rows 32-47
    e1 = np.zeros((48, 8, 128), np.float16)
    for t in range(8):
        for dl in range(128):
            e1[2 * t + dl // 64, t, dl] = 1.0
            e1[32 + 2 * t + dl // 64, t, dl] = 1.0
    w["e1"] = np.ascontiguousarray(e1.reshape(48, 8 * 128))

    w["esc"] = inp["embed_norm_scale"].reshape(1, 8 * 128).astype(np.float16)

    cq = np.empty((NL, 128, L), np.float16); sq = np.empty((NL, 128, L), np.float16)
    ck = np.empty((NL, 128, L), np.float16); sk = np.empty((NL, 128, L), np.float16)
    for l in range(NL):
        cq[l], sq[l] = _rope_tables(inp["q_norm_scale"][l])
        ck[l], sk[l] = _rope_tables(inp["k_norm_scale"][l])
    w["cq"] = cq; w["sq"] = sq; w["ck"] = ck; w["sk"] = sk
    b16 = np.zeros((128, 8 * 16), np.float16)
    for t in range(8):
        for dl in range(128):
            b16[dl, t * 16 + 2 * t + dl // 64] = 1.0 / 64.0
    w["b16"] = b16
    psw = np.zeros((128, 128), np.float16)
    for i in range(128):
        psw[i ^ 1, i] = 1.0
    w["psw"] = psw
    return w


def _patchify(frames_b):
    # (T, C, RES, RES) -> (L, PD)
    h = RES // P
    x = frames_b.reshape(T, C, h, P, h, P)
    x = x.transpose(0, 2, 4, 1, 3, 5).reshape(T * h * h, C * P * P)
    return x


def _unpatchify(tokens):
    # (L, PD) -> (T, C, RES, RES)
    h = RES // P
    y = tokens.reshape(T, h, h, C, P, P)
    return y.transpose(0, 3, 1, 4, 2, 5).reshape(T, C, RES, RES)


# ----------------------------------------------------------------------
# device kernel
# ----------------------------------------------------------------------

def _build(nl=NL):
    nc = _Bacc()
    d = {}
    d["x0T"] = nc.dram_tensor("x0T", [PDP, L], FP16, kind="ExternalInput")
    d["qk_t"] = nc.dram_tensor("qk_t", [NL, 16, 128, D], FP16, kind="ExternalInput")
    d["wv_p"] = nc.dram_tensor("wv_p", [NL, KT, 128, D], FP16, kind="ExternalInput")
    d["wo_t"] = nc.dram_tensor("wo_t", [NL, 8, 128, D], FP16, kind="ExternalInput")
    d["gu_t"] = nc.dram_tensor("gu_t", [NL, 44, 128, D], FP16, kind="ExternalInput")
    d["dn_t"] = nc.dram_tensor("dn_t", [NL, 8, 128, IHP], FP16, kind="ExternalInput")
    d["pe_t"] = nc.dram_tensor("pe_t", [8, 128, PDP], FP16, kind="ExternalInput")
    d["hd_t"] = nc.dram_tensor("hd_t", [2, 128, D], FP16, kind="ExternalInput")
    d["e1"] = nc.dram_tensor("e1", [48, 8 * 128], FP16, kind="ExternalInput")
    d["esc"] = nc.dram_tensor("esc", [1, 8 * 128], FP16, kind="ExternalInput")
    d["cq"] = nc.dram_tensor("cq", [NL, 128, L], FP16, kind="ExternalInput")
    d["sq"] = nc.dram_tensor("sq", [NL, 128, L], FP16, kind="ExternalInput")
    d["ck"] = nc.dram_tensor("ck", [NL, 128, L], FP16, kind="ExternalInput")
    d["sk"] = nc.dram_tensor("sk", [NL, 128, L], FP16, kind="ExternalInput")
    d["b16"] = nc.dram_tensor("b16", [128, 8 * 16], FP16, kind="ExternalInput")
    d["psw"] = nc.dram_tensor("psw", [128, 128], FP16, kind="ExternalInput")
    out_d = nc.dram_tensor("out", [PD, L], FP32, kind="ExternalOutput")

    with TileContext(nc) as tc:
        _emit(nc, tc, d, out_d, nl)
    nc.compile()
    return nc


def _emit(nc, tc, d, out_d, nl):
    import contextlib
    ctx = contextlib.ExitStack()
    with ctx:
        cpool = ctx.enter_context(tc.tile_pool(name="consts", bufs=1))
        xpool = ctx.enter_context(tc.tile_pool(name="x", bufs=1))
        wpool = ctx.enter_context(tc.tile_pool(name="w", bufs=3))
        apool = ctx.enter_context(tc.tile_pool(name="act", bufs=1))
        spool = ctx.enter_context(tc.tile_pool(name="small", bufs=2))
        # psum: tag "g" slots are 2 banks wide ([128,1024] fp32) so the
        # attention scores for one head-pair half (kt0+kt1+kt3 packed) fit
        # one slot; general [128,512] psums use half a slot.  2(g)*2 + 2(acc)
        # + 2(bc) = 8 banks exactly.
        ps_g = ctx.enter_context(tc.tile_pool(name="psg", bufs=4, space="PSUM"))
        ps_acc = ctx.enter_context(tc.tile_pool(name="psacc", bufs=2, space="PSUM"))
        ps_bc = ctx.enter_context(tc.tile_pool(name="psbc", bufs=2, space="PSUM"))

        # ---- persistent constants ----
        b16 = cpool.tile([128, 128], FP16, name="b16")
        nc.sync.dma_start(b16[:], d["b16"][:])
        pswc = cpool.tile([128, 128], FP16, name="pswc")
        nc.sync.dma_start(pswc[:], d["psw"][:])
        e1s = cpool.tile([48, 8 * 128], FP16, name="e1s")
        nc.sync.dma_start(e1s[:], d["e1"][:])
        escs = cpool.tile([1, 8 * 128], FP16, name="escs")
        nc.sync.dma_start(escs[:], d["esc"][:])
        ones1 = cpool.tile([1, 128], FP16, name="ones1")
        nc.gpsimd.memset(ones1[:], 1.0)
        o1c = cpool.tile([33, 64], FP16, name="o1c")
        nc.gpsimd.memset(o1c[:], 1.0)
        onesd = cpool.tile([128, 1], FP16, name="onesd")
        nc.gpsimd.memset(onesd[:], 1.0 / 1024.0)
        ep9 = cpool.tile([1, 16], FP16, name="ep9")
        nc.gpsimd.memset(ep9[:], EP9)
        epsb = cpool.tile([128, 1], FP32, name="epsb")
        nc.gpsimd.memset(epsb[:], EPS)
        identf = cpool.tile([1, 1], FP32, name="identf")
        nc.gpsimd.memset(identf[:], 1.0)

        # residual stream
        xs = [xpool.tile([128, L], FP32, name=f"x{t}") for t in range(8)]
        x16 = [xpool.tile([128, L], FP16, name=f"m{t}") for t in range(8)]

        def rms_recip(mean_ps, name, dtype=FP16):
            """[1, L] psum mean -> [1, L] rsqrt(mean+eps) via Ln+Exp."""
            lnm = spool.tile([1, L], FP32, name=f"lnm_{name}", tag="r32")
            nc.scalar.activation(lnm[:], mean_ps[:], F.Ln, bias=epsb[0:1])
            r = spool.tile([1, L], dtype, name=f"r_{name}", tag="r16")
            nc.scalar.activation(r[:], lnm[:], F.Exp, scale=-0.5)
            return r

        def emit_mean(src16, t, mean_ps, name):
            """accumulate mean(src16^2) over feature tiles into [1, L] psum."""
            sq = spool.tile([128, L], FP16, name=f"sq_{name}_{t}", tag="sq",
                            bufs=3)
            nc.vector.tensor_mul(sq[:], src16[:], src16[:])
            nc.tensor.matmul(mean_ps[:], onesd[:], sq[:],
                             start=(t == 0), stop=(t == 7))

        # ---------- patch embed ----------
        x0 = apool.tile([128, 2, L], FP16, name="x0")
        nc.sync.dma_start(x0[:], d["x0T"].rearrange("(k p) t -> p k t", p=128))
        mean_e = ps_acc.tile([1, L], FP32, name="mean_e", tag="acc")
        for t in range(8):
            wt = wpool.tile([128, PDP], FP16, name=f"pe_w{t}", tag="w")
            nc.sync.dma_start(wt[:], d["pe_t"][t])
            ps = ps_g.tile([128, L], FP32, name=f"pe_ps{t}", tag="g")
            for k in range(2):
                nc.tensor.matmul(ps[:], wt[:, k * 128:(k + 1) * 128],
                                 x0[:, k, :], start=(k == 0), stop=(k == 1))
            nc.scalar.activation(xs[t][:], ps[:], F.Copy)
            nc.vector.tensor_copy(x16[t][:], xs[t][:])
            sqe = spool.tile([128, L], FP16, name=f"sqe_{t}", tag="sq", bufs=3)
            nc.scalar.activation(sqe[:], ps[:], F.Square)
            nc.tensor.matmul(mean_e[:], onesd[:], sqe[:],
                             start=(t == 0), stop=(t == 7))
        re_sb = rms_recip(mean_e[:], "emb")
        mean_in = ps_acc.tile([1, L], FP32, name="mean_l0", tag="acc")
        for t in range(8):
            bc = ps_bc.tile([128, L], FP32, name=f"bc_emb_{t}", tag="bc")
            nc.tensor.matmul(bc[:], escs[0:1, t * 128:(t + 1) * 128], re_sb[:],
                             start=True, stop=True)
            nc.vector.tensor_mul(xs[t][:], xs[t][:], bc[:])
            nc.vector.tensor_copy(x16[t][:], xs[t][:])
            emit_mean(x16[t][:], t, mean_in[:], "l0")

        # ---------- layers ----------
        for l in range(nl):
            mean_in = _layer(nc, tc, d, l, xs, x16, cpool, wpool, apool, spool,
                             ps_g, ps_acc, ps_bc,
                             b16, e1s, ones1, o1c, onesd, ep9, epsb, identf,
                             rms_recip, mean_in, emit_mean, pswc)

        # ---------- final norm + head ----------
        rf_sb = rms_recip(mean_in[:], "fin")
        hN = [apool.tile([128, L], FP16, name=f"hN{t}", tag=f"h2{t}")
              for t in range(8)]
        bcf = ps_bc.tile([128, L], FP32, name="bc_fin", tag="bc")
        nc.tensor.matmul(bcf[:], ones1[:], rf_sb[:], start=True, stop=True)
        bcfh = spool.tile([128, L], FP16, name="bcfh", tag="bchh", bufs=1)
        nc.vector.tensor_copy(bcfh[:], bcf[:])
        for t in range(8):
            nc.vector.tensor_mul(hN[t][:], x16[t][:], bcfh[:])
        for o in range(2):
            wt = wpool.tile([128, D], FP16, name=f"hd_w{o}", tag="w")
            nc.sync.dma_start(wt[:], d["hd_t"][o])
            ps = ps_g.tile([128, L], FP32, name=f"hd_ps{o}", tag="g")
            for k in range(KT):
                nc.tensor.matmul(ps[:], wt[:, k * 128:(k + 1) * 128],
                                 hN[k][:], start=(k == 0), stop=(k == KT - 1))
            rows = 128 if o == 0 else PD - 128
            ot = apool.tile([128, L], FP32, name=f"hd_o{o}")
            nc.scalar.activation(ot[:rows, :], ps[:rows, :], F.Copy)
            nc.sync.dma_start(out_d[o * 128:o * 128 + rows, :], ot[:rows, :])


def _layer(nc, tc, d, l, xs, x16, cpool, wpool, apool, spool,
           ps_g, ps_acc, ps_bc,
           b16, e1s, ones1, o1c, onesd, ep9, epsb, identf, rms_recip,
           mean_in, emit_mean, pswc):
    # per-layer rope/scale tables (prefetched; DMA engine is idle)
    rtab = spool.tile([128, 4, L], FP16, name=f"rtab_{l}", tag="rtab", bufs=2)
    nc.sync.dma_start(rtab[:, 0, :], d["cq"][l])
    nc.sync.dma_start(rtab[:, 1, :], d["sq"][l])
    nc.sync.dma_start(rtab[:, 2, :], d["ck"][l])
    nc.sync.dma_start(rtab[:, 3, :], d["sk"][l])

    # mean1-derived values (mean_in accumulated at the previous layer's tail)
    zs = spool.tile([1, L], FP16, name=f"zs_{l}", tag="zs")
    nc.vector.tensor_scalar_mul(zs[:], mean_in[:], EPS / EP9)
    r1 = rms_recip(mean_in[:], f"r1_{l}", dtype=FP32)
    rT = spool.tile([128, 4], FP32, name=f"rT_{l}", tag="rT")

    # msq accumulators: q at psum partitions 0-15 (bank 1), k at partitions
    # 32-47 (bank 2) so the accumulation matmuls pair up on disjoint PE
    # column groups and run concurrently.
    msq_q = ps_acc.tile([16, L], FP32, name=f"msqq_{l}", tag="acc")
    msq_k = ps_acc.tile([48, L], FP32, name=f"msqk_{l}", tag="acc")

    # ---- Phase A: qkv q/k GEMM + statistics, tensor queue kept dense ----
    qraw = []
    for grp in range(4):
        if grp == 1:
            # transpose r1 -> rT [128, 4] (per-token scale for token-major
            # v); emitted behind the first qkv chains so the tensor queue
            # head never blocks on the scalar r1 chain.
            trp = ps_bc.tile([128, 4], FP32, name=f"trp_{l}", tag="bc")
            for b in range(4):
                nc.tensor.transpose(trp[:, b:b + 1],
                                    r1[:, b * 128:(b + 1) * 128], identf[:])
            nc.vector.tensor_copy(rT[:], trp[:])
        wt = wpool.tile([128, 4, D], FP16, name=f"qkw_{l}_{grp}", tag="w")
        nc.sync.dma_start(wt[:], d["qk_t"][l, grp * 4:(grp + 1) * 4]
                          .rearrange("g p n -> p g n"))
        for gi in range(4):
            ot = grp * 4 + gi
            ps = ps_g.tile([128, L], FP32, name=f"qk_ps_{l}_{ot}", tag="g")
            for k in range(KT):
                nc.tensor.matmul(ps[:], wt[:, gi, k * 128:(k + 1) * 128],
                                 x16[k][:], start=(k == 0), stop=(k == KT - 1))
            qr = spool.tile([128, L], FP16, name=f"qraw_{l}_{ot}", tag="qraw",
                            bufs=16)
            nc.vector.tensor_copy(qr[:], ps[:])
            qraw.append(qr)
            sq = spool.tile([128, L], FP16, name=f"qsq_{l}_{ot}", tag="sq",
                            bufs=3)
            nc.vector.tensor_mul(sq[:], qr[:], qr[:])
            tt = ot % 8
            if ot < 8:
                nc.tensor.matmul(msq_q[:], b16[:, tt * 16:(tt + 1) * 16],
                                 sq[:], start=(tt == 0), stop=False,
                                 tile_position=(0, 0))
            else:
                nc.tensor.matmul(msq_k[32:48, :], b16[:, tt * 16:(tt + 1) * 16],
                                 sq[:], start=(tt == 0), stop=False,
                                 tile_position=(0, 32))

    # eps correction: msq += ep9 * zs  (= eps * ir2); 2-way packed
    nc.tensor.matmul(msq_q[:], ep9[:], zs[:], start=False, stop=True,
                     tile_position=(0, 0))
    nc.tensor.matmul(msq_k[32:48, :], ep9[:], zs[:], start=False, stop=True,
                     tile_position=(0, 32))

    # per-head q/k norm multipliers: alf rows 0-15 = q, rows 32-47 = k
    alf = spool.tile([48, L], FP16, name=f"alf_{l}", tag="alf")
    for row, msq in ((0, msq_q[:]), (32, msq_k[32:48, :])):
        tl = spool.tile([16, L], FP32, name=f"aln_{l}_{row}", tag="a32")
        nc.scalar.activation(tl[:], msq, F.Ln)
        nc.scalar.activation(alf[row:row + 16, :], tl[:], F.Exp, scale=-0.5)

    # ---- Phase B: v GEMM + rope, interleaved ----
    vsb = [apool.tile([128, 16 * 65], FP16, name=f"vsb_{l}_{b}", tag=f"vsb{b}")
           for b in range(4)]
    for b in range(4):
        nc.gpsimd.memset(
            vsb[b][:].rearrange("p (h c) -> p h c", c=65)[:, :, 64:65], 1.0)
    wva = wpool.tile([128, 4, D], FP16, name=f"vwa_{l}", tag="w")
    nc.sync.dma_start(wva[:], d["wv_p"][l, 0:4].rearrange("k p n -> p k n"))
    wvb = wpool.tile([128, 4, D], FP16, name=f"vwb_{l}", tag="w")
    nc.sync.dma_start(wvb[:], d["wv_p"][l, 4:8].rearrange("k p n -> p k n"))

    qf = [None] * 16

    def emit_rope(ot):
        t = ot % 8
        arow = 0 if ot < 8 else 32
        ci, si = (0, 1) if ot < 8 else (2, 3)
        bc = ps_bc.tile([128, L], FP32, name=f"rbc_{l}_{ot}", tag="bc")
        nc.tensor.matmul(bc[:], e1s[arow:arow + 16, t * 128:(t + 1) * 128],
                         alf[arow:arow + 16, :], start=True, stop=True)
        sw = spool.tile([128, L], FP16, name=f"rsw_{l}_{ot}", tag="u2")
        if DBG_NO_SHUF:
            swp = ps_bc.tile([128, L], FP32, name=f"rswp_{l}_{ot}", tag="bc")
            nc.tensor.matmul(swp[:], pswc[:], qraw[ot][:], start=True,
                             stop=True)
            nc.vector.tensor_copy(sw[:], swp[:])
        else:
            nc.vector.stream_shuffle(sw[:], qraw[ot][:], SWAP_MASK)
        u1 = spool.tile([128, L], FP16, name=f"u1_{l}_{ot}", tag="u1")
        nc.vector.tensor_mul(u1[:], qraw[ot][:], rtab[:, ci, :])
        u2 = spool.tile([128, L], FP16, name=f"u2_{l}_{ot}", tag="u1")
        nc.vector.tensor_mul(u2[:], sw[:], rtab[:, si, :])
        nc.vector.tensor_add(u1[:], u1[:], u2[:])
        qt = apool.tile([128, L], FP16, name=f"qf_{l}_{ot}", tag=f"qf{ot}")
        nc.vector.tensor_mul(qt[:], u1[:], bc[:])
        qf[ot] = qt

    def emit_v(b, n):
        ps = ps_g.tile([128, 512], FP32, name=f"v_ps_{l}_{b}_{n}", tag="g")
        for k in range(KT):
            wv = wva if k < 4 else wvb
            nc.tensor.matmul(ps[:], x16[k][:, b * 128:(b + 1) * 128],
                             wv[:, k % 4, n * 512:(n + 1) * 512],
                             start=(k == 0), stop=(k == KT - 1))
        dst = vsb[b][:].rearrange("p (h c) -> p h c", c=65)[:, n * 8:(n + 1) * 8, 0:64]
        nc.vector.tensor_scalar_mul(dst, ps[:], rT[:, b:b + 1])

    # ---- Phase C: attention, software-pipelined with v/rope as filler ----
    oun = [apool.tile([128, L], FP16, name=f"oun_{l}_{t}", tag=f"oun{t}")
           for t in range(8)]
    of = [spool.tile([128, L], FP16, name=f"of_{l}_{t}", tag=f"of{t}", bufs=1)
          for t in range(8)]
    est_store = {}

    def emit_scores(ti):
        tiles = []
        for kt in range(4):
            q0 = 128 * kt
            sta = ps_g.tile([128, L], FP32, name=f"st_{l}_{ti}a_{kt}", tag="g")
            stb = ps_g.tile([128, L], FP32, name=f"st_{l}_{ti}b_{kt}", tag="g")
            nc.tensor.matmul(sta[:, q0:], qf[8 + ti][0:64, kt * 128:(kt + 1) * 128],
                             qf[ti][0:64, q0:], start=True, stop=True)
            nc.tensor.matmul(stb[:, q0:], qf[8 + ti][64:128, kt * 128:(kt + 1) * 128],
                             qf[ti][64:128, q0:], start=True, stop=True)
            esta = spool.tile([128, L], FP16, name=f"est_{l}_{ti}a_{kt}",
                              tag="esta", bufs=12)
            estb = spool.tile([128, L], FP16, name=f"est_{l}_{ti}b_{kt}",
                              tag="estb", bufs=12)
            nc.scalar.activation(esta[:, q0:], sta[:, q0:], F.Exp, scale=0.125)
            nc.scalar.activation(estb[:, q0:], stb[:, q0:], F.Exp, scale=0.125)
            nc.gpsimd.memset(esta[64:128, q0:q0 + 64], 0.0)
            nc.gpsimd.memset(estb[64:128, q0:q0 + 64], 0.0)
            tiles.append((esta, estb))
        est_store[ti] = tiles

    def emit_av(ti):
        ha, hb = 2 * ti, 2 * ti + 1
        oea = ps_acc.tile([65, L], FP32, name=f"oe_{l}_{ha}", tag="acc")
        oeb = ps_acc.tile([65, L], FP32, name=f"oe_{l}_{hb}", tag="acc")
        tiles = est_store.pop(ti)
        for kt in range(4):
            q0 = 128 * kt
            esta, estb = tiles[kt]
            nc.tensor.matmul(oea[:, q0:], vsb[kt][:, ha * 65:(ha + 1) * 65],
                             esta[:, q0:], start=(kt == 0), stop=(kt == 3))
            nc.tensor.matmul(oeb[:, q0:], vsb[kt][:, hb * 65:(hb + 1) * 65],
                             estb[:, q0:], start=(kt == 0), stop=(kt == 3))
        # denominator reciprocal via exp(-ln(den)) on [33, L] staging (both
        # functions live in the resident natlog_exp table set)
        den = spool.tile([33, L], FP32, name=f"den_{l}_{ti}", tag="den",
                         bufs=2)
        nc.gpsimd.memset(den[:], 1.0)
        nc.vector.tensor_copy(den[0:1, :], oea[64:65, :])
        nc.vector.tensor_copy(den[32:33, :], oeb[64:65, :])
        lnd = spool.tile([33, L], FP32, name=f"lnd_{l}_{ti}", tag="lnd",
                         bufs=1)
        nc.scalar.activation(lnd[:], den[:], F.Ln)
        rden = spool.tile([33, L], FP16, name=f"rden_{l}_{ti}", tag="rden",
                          bufs=2)
        nc.scalar.activation(rden[:], lnd[:], F.Exp, scale=-1.0)
        nc.vector.tensor_copy(oun[ti][0:64, :], oea[0:64, :])
        nc.vector.tensor_copy(oun[ti][64:128, :], oeb[0:64, :])
        # broadcast 1/den over partitions; 2-way packed (rows 0-63 / 64-127)
        bc = ps_bc.tile([128, L], FP32, name=f"nbc_{l}_{ti}", tag="bc")
        nc.tensor.matmul(bc[0:64, :], o1c[0:1, :], rden[0:1, :],
                         start=True, stop=True, tile_position=(0, 0))
        nc.tensor.matmul(bc[64:128, :], o1c[32:33, :], rden[32:33, :],
                         start=True, stop=True, tile_position=(32, 64))
        nc.vector.tensor_mul(of[ti][:], oun[ti][:], bc[:])

    # v + rope first (dense filler while alpha/stats chains settle), then
    # the score/attn-v pipeline with a 2-stage exp lead.
    rope_order = [x for p in zip(range(8), range(8, 16)) for x in p]
    vchunks = [(b, n) for n in range(2) for b in range(4)]
    for i in range(8):
        emit_rope(rope_order[2 * i])
        emit_rope(rope_order[2 * i + 1])
        emit_v(*vchunks[i])
    emit_scores(0)
    emit_scores(1)
    for ti in range(2, 8):
        emit_av(ti - 2)
        emit_scores(ti)
    emit_av(6)
    emit_av(7)

    # ---- Phase D: out projection + residual + mean2 ----
    mean2 = ps_acc.tile([1, L], FP32, name=f"mean2_{l}", tag="acc")
    for grp in range(2):
        wt = wpool.tile([128, 4, D], FP16, name=f"wo_{l}_{grp}", tag="w")
        nc.sync.dma_start(wt[:], d["wo_t"][l, grp * 4:(grp + 1) * 4]
                          .rearrange("g p n -> p g n"))
        for gi in range(4):
            t = grp * 4 + gi
            ps = ps_g.tile([128, L], FP32, name=f"xa_ps_{l}_{t}", tag="g")
            for k in range(KT):
                nc.tensor.matmul(ps[:], wt[:, gi, k * 128:(k + 1) * 128],
                                 of[k][:], start=(k == 0), stop=(k == KT - 1))
            nc.vector.scalar_tensor_tensor(x16[t][:], ps[:], 0.0, xs[t][:],
                                           ALU.add, ALU.add)
            nc.vector.tensor_add(xs[t][:], xs[t][:], ps[:])
            emit_mean(x16[t][:], t, mean2[:], f"m2_{l}")

    # ---- Phase E: MLP ----
    r2 = rms_recip(mean2[:], f"r2_{l}")
    h2 = [apool.tile([128, L], FP16, name=f"h2_{l}_{t}", tag=f"h2{t}")
          for t in range(8)]
    bch = ps_bc.tile([128, L], FP32, name=f"bch_{l}", tag="bc")
    nc.tensor.matmul(bch[:], ones1[:], r2[:], start=True, stop=True)
    bchh = spool.tile([128, L], FP16, name=f"bchh_{l}", tag="bchh", bufs=1)
    nc.vector.tensor_copy(bchh[:], bch[:])
    for t in range(8):
        nc.vector.tensor_mul(h2[t][:], x16[t][:], bchh[:])

    pj = []
    for j in range(JT):
        wt = wpool.tile([128, 4, D], FP16, name=f"gu_{l}_{j}", tag="w")
        nc.sync.dma_start(wt[:], d["gu_t"][l, j * 4:(j + 1) * 4]
                          .rearrange("g p n -> p g n"))
        # consume each psum right after its group (ps_g rotates 2 slots)
        sg1 = spool.tile([128, L], FP16, name=f"sg1_{l}_{j}", tag="sg1")
        sg2 = spool.tile([128, L], FP16, name=f"sg2_{l}_{j}", tag="sg2")
        ta = spool.tile([128, L], FP16, name=f"ta_{l}_{j}", tag="ta")
        tb = spool.tile([128, L], FP16, name=f"tb_{l}_{j}", tag="tb")
        for gi in range(4):
            ps = ps_g.tile([128, L], FP32, name=f"gu_ps_{l}_{j}_{gi}", tag="g")
            for k in range(KT):
                nc.tensor.matmul(ps[:], wt[:, gi, k * 128:(k + 1) * 128],
                                 h2[k][:], start=(k == 0), stop=(k == KT - 1))
            if gi == 0:
                nc.scalar.activation(sg1[:], ps[:], F.Sigmoid if SIM_SAFE else F.Silu)
            elif gi == 1:
                nc.scalar.activation(sg2[:], ps[:], F.Sigmoid if SIM_SAFE else F.Silu)
            elif gi == 2:
                nc.vector.scalar_tensor_tensor(ta[:], ps[:], 0.0, sg1[:],
                                               ALU.add, ALU.mult)
            else:
                nc.vector.scalar_tensor_tensor(tb[:], ps[:], 0.0, sg2[:],
                                               ALU.add, ALU.mult)
        p = spool.tile([128, L], FP16, name=f"p_{l}_{j}", tag=f"p{j}", bufs=1)
        nc.vector.tensor_add(p[:], ta[:], tb[:])
        pj.append(p)

    mean_next = ps_acc.tile([1, L], FP32, name=f"mean_{l + 1}", tag="acc")
    for grp in range(4):
        wt = wpool.tile([128, 2, IHP], FP16, name=f"dn_{l}_{grp}", tag="w")
        nc.sync.dma_start(wt[:], d["dn_t"][l, grp * 2:(grp + 1) * 2]
                          .rearrange("g p n -> p g n"))
        for gi in range(2):
            t = grp * 2 + gi
            ps = ps_g.tile([128, L], FP32, name=f"dn_ps_{l}_{t}", tag="g")
            for j in range(JT):
                nc.tensor.matmul(ps[:], wt[:, gi, j * 128:(j + 1) * 128],
                                 pj[j][:], start=(j == 0), stop=(j == JT - 1))
            nc.vector.scalar_tensor_tensor(x16[t][:], ps[:], 0.0, xs[t][:],
                                           ALU.add, ALU.add)
            nc.vector.tensor_add(xs[t][:], xs[t][:], ps[:])
            emit_mean(x16[t][:], t, mean_next[:], f"mn_{l}")
    return mean_next


# ----------------------------------------------------------------------
# entry point
# ----------------------------------------------------------------------

def _get_nc(nl=NL):
    if nl not in _CACHE:
        _CACHE[nl] = _build(nl)
    return _CACHE[nl]


def run(inputs, nl=NL, trace=False):
    inputs = {k: np.asarray(v) for k, v in inputs.items()}
    w = _prep_weights(inputs)
    in_maps = []
    for b in range(N_CORES):
        tok = _patchify(inputs["frames"][b]).astype(np.float32)
        x0T = np.zeros((PDP, L), np.float16)
        x0T[:PD] = tok.T.astype(np.float16)
        m = {"x0T": x0T, "qk_t": w["qk_t"], "wv_p": w["wv_p"],
             "wo_t": w["wo_t"], "gu_t": w["gu_t"], "dn_t": w["dn_t"],
             "pe_t": w["pe_t"], "hd_t": w["hd_t"],
             "e1": w["e1"], "esc": w["esc"],
             "cq": w["cq"], "sq": w["sq"], "ck": w["ck"], "sk": w["sk"],
             "b16": w["b16"], "psw": w["psw"]}
        in_maps.append(m)
    nc = _get_nc(nl)
    res = run_bass_kernel_spmd(nc, in_maps, list(range(N_CORES)), trace=trace)
    outs = []
    for b in range(N_CORES):
        tok = res.results[b]["out"].T  # (L, PD)
        outs.append(_unpatchify(tok))
    return np.stack(outs).astype(np.float32), res


def kernel(**inputs) -> np.ndarray:
    out, _ = run(inputs)
    return out

